# revision 1
# baseline (speedup 1.0000x reference)
"""DigitCaps (CapsNet dynamic routing) kernel for 8 Trainium2 NeuronCores.

Reference math:
  u_hat[b,r,c,o] = sum_i W[r,c,o,i] * x[b,r,i]
  b_ij = 0;  3 routing iterations:
     c = softmax_r(b);  s[b,c,o] = sum_r c[r,c] u_hat[b,r,c,o];
     v = squash(s);     b += mean_b(sum_o u_hat[b,r,c,o] v[b,c,o])
  returns v[..., None]  (256, 10, 16, 1)

Strategy: data-parallel over batch (32 per core), W replicated.  u_hat
(189 MB) is NEVER materialized — the routing coefficients are folded into
the weights so every pass is a dense matmul over the contraction dim
K=(i,r)=9216:
    s-matmul:  s[b,(c,o)]   = sum_K  XT[K,b] * (c-scaled Wg)[K,(c,o)]
    G-matmul:  G[K,(c,o)]   = sum_b  xn[b,K] * (v[b,(c,o)]/B)
    agreement: P = Wg .* G;  abar[r,c] = sum_{i,o} P
Rows are ordered (i, r) with r-major tiles of 128, so partition p of tile
u=(i,T) holds r = T*128+p.  That makes the o-reduction of P a tree of
packed bf16 adds (innermost o), the i-reduction a free-axis DVE reduce,
the softmax a partition_all_reduce + free-axis reduce, and the c-fold a
packed bf16 broadcast multiply (no indicator matmuls anywhere).  abar is
AllReduce-summed across the 8 cores each iteration.  Matmuls run in bf16;
softmax and squash in fp32.  Host-side numpy prepares all SBUF layouts
partition-major so DMAs are few large contiguous descriptors.
"""
import sys
if '/opt/trn_rl_repo' not in sys.path:
    sys.path.insert(0, '/opt/trn_rl_repo')
import numpy as np
import ml_dtypes

import concourse.bass as bass
import concourse.bacc as bacc
import concourse.mybir as mybir
import concourse.tile as tile
from concourse import bass_utils
from concourse import bass_isa

BF16 = mybir.dt.bfloat16
F32 = mybir.dt.float32
ALU = mybir.AluOpType
ACT = mybir.ActivationFunctionType

B, R, C, O, I = 256, 1152, 10, 16, 8
NCORES = 8
BL = B // NCORES          # 32 local batch
RT = 9                    # r tiles of 128 (per i)
NT = 72                   # (i, r) tiles of 128: u = i*RT + T
CO = C * O                # 160, free order (c,o): idx = c*O + o
NITER = 3

_CACHE = {}


def _build(n_cores=NCORES, repeat=1, use_collective=True):
    nc = bacc.Bacc("TRN2", target_bir_lowering=False, debug=False,
                   num_devices=n_cores)
    wg_d = nc.dram_tensor("wg", [128, NT * CO], BF16, kind="ExternalInput")
    xt_d = nc.dram_tensor("xt", [128, NT * BL], BF16, kind="ExternalInput")
    xn_d = nc.dram_tensor("xn", [BL, NT * 128], BF16, kind="ExternalInput")
    out_d = nc.dram_tensor("out", [BL, CO], F32, kind="ExternalOutput")

    with tile.TileContext(nc) as tc:
        with (
            tc.tile_pool(name="big", bufs=1) as big,
            tc.tile_pool(name="small", bufs=1) as small,
            tc.tile_pool(name="sps", bufs=1, space="PSUM") as sps,
            tc.tile_pool(name="gps", bufs=4, space="PSUM") as gps,
            tc.tile_pool(name="dram", bufs=4, space="DRAM") as dram,
        ):
            Wg = big.tile([128, NT * CO], BF16, tag="Wg")
            Wp = big.tile([128, NT * CO], BF16, tag="Wp")
            XT = big.tile([128, NT * BL], BF16, tag="XT")
            XN = big.tile([BL, NT * 128], BF16, tag="XN")
            Gb = big.tile([128, NT * CO], BF16, tag="Gb")
            T1 = big.tile([128, NT * C * 8], BF16, tag="T1")
            T2 = big.tile([128, NT * C * 4], BF16, tag="T2")
            T3 = big.tile([128, NT * C * 2], BF16, tag="T3")
            Q = big.tile([128, NT * C], F32, tag="Q")

            b_sb = small.tile([128, RT * C], F32, tag="b")
            expb = small.tile([128, RT * C], F32, tag="expb")
            esum = small.tile([128, RT * C], F32, tag="esum")
            c_sb = small.tile([128, RT * C], F32, tag="c")
            c16 = small.tile([128, RT * C], BF16, tag="c16")
            crep = small.tile([128, RT * C * O], BF16, tag="crep")
            abar = small.tile([128, RT * C], F32, tag="abar")
            arr = small.tile([128, RT * C], F32, tag="arr")
            zp = small.tile([128, C], F32, tag="zp")
            zr = small.tile([128, C], F32, tag="zr")
            se = small.tile([BL, CO], F32, tag="se")
            ab = small.tile([BL, CO], F32, tag="ab")
            sq = small.tile([BL, CO], F32, tag="sq")
            rd = small.tile([BL, CO], F32, tag="rd")
            num = small.tile([BL, CO], F32, tag="num")
            vv = small.tile([BL, CO], F32, tag="v")
            vbf = small.tile([BL, CO], BF16, tag="vbf")

            for _rep in range(repeat):
                nc.sync.dma_start(out=XT[:, :], in_=xt_d[:, :])
                NCH = NT // 4
                for ch in range(4):
                    sl = slice(ch * NCH * CO, (ch + 1) * NCH * CO)
                    nc.sync.dma_start(out=Wg[:, sl], in_=wg_d[:, sl])
                nc.sync.dma_start(out=XN[:, :], in_=xn_d[:, :])
                nc.vector.memset(b_sb[:, :], 0.0)

                for k in range(NITER):
                    if k > 0:
                        # c = softmax over r (partitions x RT tiles)
                        nc.scalar.activation(expb[:, :], b_sb[:, :], ACT.Exp)
                        nc.gpsimd.partition_all_reduce(
                            esum[:, :], expb[:, :], channels=128,
                            reduce_op=bass_isa.ReduceOp.add)
                        nc.vector.tensor_reduce(
                            zp[:, :],
                            esum[:, :].rearrange("p (T c) -> p c T", c=C),
                            axis=mybir.AxisListType.X, op=ALU.add)
                        nc.vector.reciprocal(zr[:, :], zp[:, :])
                        nc.vector.tensor_tensor(
                            c_sb[:, :].rearrange("p (T c) -> p T c", c=C),
                            expb[:, :].rearrange("p (T c) -> p T c", c=C),
                            zr[:, :].unsqueeze(1).broadcast_to((128, RT, C)),
                            op=ALU.mult)
                        nc.scalar.activation(c16[:, :], c_sb[:, :], ACT.Copy)
                        # crep[p,(T,c,o)] = c16[p,(T,c)] replicated over o
                        nc.vector.tensor_copy(
                            crep[:, :].rearrange("p (T c o) -> p T c o",
                                                 c=C, o=O),
                            c16[:, :].rearrange("p (T c) -> p T c", c=C)
                            .unsqueeze(3).broadcast_to((128, RT, C, O)))
                        # W' = Wg * crep  (packed bf16, broadcast over i)
                        nc.vector.tensor_tensor(
                            Wp[:, :].rearrange("p (i f) -> p i f",
                                               f=RT * C * O),
                            Wg[:, :].rearrange("p (i f) -> p i f",
                                               f=RT * C * O),
                            crep[:, :].unsqueeze(1).broadcast_to(
                                (128, I, RT * C * O)),
                            op=ALU.mult)

                    # s matmul over K = 9216 (72 tiles)
                    mov = Wg if k == 0 else Wp
                    s_ps = sps.tile([BL, CO], F32, tag="s")
                    for u in range(NT):
                        nc.tensor.matmul(s_ps[:, :],
                                         XT[:, u * BL:(u + 1) * BL],
                                         mov[:, u * CO:(u + 1) * CO],
                                         start=(u == 0), stop=(u == NT - 1))
                    # squash: v = s*|s| / (1+s^2)
                    nc.scalar.activation(se[:, :], s_ps[:, :], ACT.Copy,
                                         scale=(1.0 / R if k == 0 else 1.0))
                    nc.scalar.activation(ab[:, :], se[:, :], ACT.Abs)
                    nc.vector.tensor_mul(sq[:, :], se[:, :], se[:, :])
                    nc.vector.tensor_scalar_add(sq[:, :], sq[:, :], 1.0)
                    nc.vector.reciprocal(rd[:, :], sq[:, :])
                    nc.vector.tensor_mul(num[:, :], se[:, :], ab[:, :])
                    nc.vector.tensor_mul(vv[:, :], num[:, :], rd[:, :])

                    if k == NITER - 1:
                        nc.sync.dma_start(out=out_d[:, :], in_=vv[:, :])
                        continue

                    nc.scalar.activation(vbf[:, :], vv[:, :], ACT.Copy,
                                         scale=1.0 / B)

                    # G matmul; PSUM drains to bf16 on the (idle) ACT engine
                    # so the P-multiply runs as packed all-bf16 DVE chunks
                    # (2x_1p mode) instead of f32-PSUM reads at 1x.
                    for g in range(NT // 3):
                        g_ps = gps.tile([128, 3 * CO], F32, tag="g")
                        for j in range(3):
                            u = 3 * g + j
                            nc.tensor.matmul(g_ps[:, j * CO:(j + 1) * CO],
                                             XN[:, u * 128:(u + 1) * 128],
                                             vbf[:, :], start=True, stop=True)
                        nc.scalar.copy(Gb[:, g * 3 * CO:(g + 1) * 3 * CO],
                                       g_ps[:, :])
                    for h in range(12):
                        sl = slice(h * 2 * 3 * CO, (h + 1) * 2 * 3 * CO)
                        nc.vector.tensor_tensor(Wp[:, sl], Wg[:, sl],
                                                Gb[:, sl], op=ALU.mult)

                    # abar[r,c] = sum_{o,i} P : packed bf16 o-tree + i-reduce
                    Pv = Wp[:, :].rearrange("p (f o) -> p f o", o=O)
                    T1v = T1[:, :].rearrange("p (f o) -> p f o", o=O // 2)
                    nc.vector.tensor_tensor(
                        T1v, Pv[:, :, 0:8], Pv[:, :, 8:16], op=ALU.add)
                    T2v = T2[:, :].rearrange("p (f o) -> p f o", o=O // 4)
                    nc.vector.tensor_tensor(
                        T2v, T1v[:, :, 0:4], T1v[:, :, 4:8], op=ALU.add)
                    T3v = T3[:, :].rearrange("p (f o) -> p f o", o=O // 8)
                    nc.vector.tensor_tensor(
                        T3v, T2v[:, :, 0:2], T2v[:, :, 2:4], op=ALU.add)
                    nc.vector.tensor_tensor(
                        Q[:, :].rearrange("p (f o) -> p f o", o=1),
                        T3v[:, :, 0:1], T3v[:, :, 1:2], op=ALU.add)
                    nc.vector.tensor_reduce(
                        abar[:, :],
                        Q[:, :].rearrange("p (i T c) -> p T c i", i=I, c=C),
                        axis=mybir.AxisListType.X, op=ALU.add)

                    # AllReduce of abar across the 8 cores (elementwise sum,
                    # so keep the raw [128, 90] SBUF layout in DRAM)
                    if use_collective:
                        ar_in = dram.tile([128, RT * C], F32, tag="arin")
                        ar_out = dram.tile([128, RT * C], F32, tag="arout")
                        nc.sync.dma_start(out=ar_in[:, :], in_=abar[:, :])
                        nc.gpsimd.collective_compute(
                            "AllReduce", ALU.add,
                            replica_groups=[list(range(n_cores))],
                            ins=[ar_in[:, :].opt()],
                            outs=[ar_out[:, :].opt()])
                        nc.sync.dma_start(out=arr[:, :], in_=ar_out[:, :])
                    else:
                        nc.vector.tensor_scalar_mul(arr[:, :], abar[:, :],
                                                    float(n_cores))
                    nc.vector.tensor_add(b_sb[:, :], b_sb[:, :], arr[:, :])

    nc.compile()
    return nc


def _host_inputs(x, W):
    # rows ordered (i, r), r-major tiles: tile u = i*RT + T holds
    # r = T*128 + p on partition p.  Free order (c, o).  Partition-major.
    wg = np.ascontiguousarray(
        W.transpose(3, 0, 1, 2).reshape(I, RT, 128, CO)
        .transpose(2, 0, 1, 3).reshape(128, NT * CO)).astype(ml_dtypes.bfloat16)
    in_maps = []
    for cidx in range(NCORES):
        xc = x[cidx * BL:(cidx + 1) * BL]          # (BL, R, I)
        xt = np.ascontiguousarray(
            xc.transpose(2, 1, 0).reshape(I, RT, 128, BL)
            .transpose(2, 0, 1, 3).reshape(128, NT * BL)).astype(
            ml_dtypes.bfloat16)
        xn = np.ascontiguousarray(
            xc.transpose(0, 2, 1).reshape(BL, NT * 128)).astype(
            ml_dtypes.bfloat16)
        in_maps.append({"wg": wg, "xt": xt, "xn": xn})
    return in_maps


def kernel(x, W):
    x = np.ascontiguousarray(np.asarray(x, dtype=np.float32))
    W = np.ascontiguousarray(np.asarray(W, dtype=np.float32))
    assert x.shape == (B, R, I) and W.shape == (R, C, O, I)
    if "nc" not in _CACHE:
        _CACHE["nc"] = _build()
    nc = _CACHE["nc"]
    in_maps = _host_inputs(x, W)
    res = bass_utils.run_bass_kernel_spmd(nc, in_maps,
                                          core_ids=list(range(NCORES)))
    vs = [r["out"].reshape(BL, C, O) for r in res.results]
    return np.concatenate(vs, axis=0)[..., None].astype(np.float32)



# revision 2
# speedup vs baseline: 1.0134x; 1.0134x over previous
"""DigitCaps (CapsNet dynamic routing) kernel for 8 Trainium2 NeuronCores.

Reference math:
  u_hat[b,r,c,o] = sum_i W[r,c,o,i] * x[b,r,i]
  b_ij = 0;  3 routing iterations:
     c = softmax_r(b);  s[b,c,o] = sum_r c[r,c] u_hat[b,r,c,o];
     v = squash(s);     b += mean_b(sum_o u_hat[b,r,c,o] v[b,c,o])
  returns v[..., None]  (256, 10, 16, 1)

Strategy: ZERO-COMMUNICATION full replication.  The routing logits b_ij are
batch-shared, so the routing trajectory is identical on every core; each
core computes it for the FULL batch (B=256) for iterations 0-1 (whose only
product is the shared b_ij update), then computes the final-iteration
capsule outputs only for its OWN 32-batch slice (per-core xto input).  No
collectives, no remote DMA, no cross-core sync of any kind.

u_hat (189 MB) is never materialized: the routing coefficients are folded
into the weights so every pass is a dense matmul over K=(i,r)=9216:
    s-matmul:  s[b,(c,o)]   = sum_K  XTF[K,b] * (c-scaled Wg)[K,(c,o)]
    G-matmul:  G[K,(c,o)]   = sum_b  XN[b,K] * (v[b,(c,o)]/B)
    agreement: abar[r,c]    = sum_{i,o} Wg .* G
Rows are ordered (i, r) with r-major tiles of 128, so partition p of tile
u=(i,T) holds r = T*128+p.  x is DMA'd in both layouts (K-major XTF for
the s-matmul, b-major XN halves for the G-matmul), chunk-interleaved in
first-use order so every pass streams behind the serial DMA device.  The
o-reduction of abar is a tree of packed bf16 adds run per u-half so it
pipelines into the G drains, the i-reduction a free-axis DVE reduce, the
softmax a partition_all_reduce + free-axis reduce, and the c-fold a packed
bf16 broadcast multiply (DVE-heavy split; Pool tensor ops model ~2.5x
slower per element).  PSUM accumulation chains each own a full bank (2 KB
zero region).  Matmuls run in bf16; softmax and squash in fp32.
"""
import sys
if '/opt/trn_rl_repo' not in sys.path:
    sys.path.insert(0, '/opt/trn_rl_repo')
import numpy as np
import ml_dtypes

import concourse.bass as bass
import concourse.bacc as bacc
import concourse.mybir as mybir
import concourse.tile as tile
from concourse import bass_utils
from concourse import bass_isa

BF16 = mybir.dt.bfloat16
F32 = mybir.dt.float32
ALU = mybir.AluOpType
ACT = mybir.ActivationFunctionType

B, R, C, O, I = 256, 1152, 10, 16, 8
NCORES = 8
BL = B // NCORES          # 32 own batch (final pass only)
RT = 9                    # r tiles of 128 (per i)
NT = 72                   # (i, r) tiles of 128: u = i*RT + T
CO = C * O                # 160, free order (c,o): idx = c*O + o
NITER = 3

_CACHE = {}


def _build(n_cores=NCORES, repeat=1):
    nc = bacc.Bacc("TRN2", target_bir_lowering=False, debug=False,
                   num_devices=n_cores)
    wg_d = nc.dram_tensor("wg", [128, NT * CO], BF16, kind="ExternalInput")
    xtf_d = nc.dram_tensor("xtf", [128, NT * B], BF16, kind="ExternalInput")
    xto_d = nc.dram_tensor("xto", [128, NT * BL], BF16, kind="ExternalInput")
    xn0_d = nc.dram_tensor("xn0", [128, NT * 128], BF16, kind="ExternalInput")
    xn1_d = nc.dram_tensor("xn1", [128, NT * 128], BF16, kind="ExternalInput")
    out_d = nc.dram_tensor("out", [80, 2 * BL], F32, kind="ExternalOutput")

    with tile.TileContext(nc) as tc:
        with (
            tc.tile_pool(name="big", bufs=1) as big,
            tc.tile_pool(name="small", bufs=1) as small,
            tc.tile_pool(name="sps", bufs=1, space="PSUM") as sps,
            tc.tile_pool(name="gps", bufs=2, space="PSUM") as gps,
        ):
            Wg = big.tile([128, NT * CO], BF16, tag="Wg")
            Wp = big.tile([128, NT * CO], BF16, tag="Wp")
            XTF = big.tile([128, NT * B], BF16, tag="XTF")
            XTO = big.tile([128, NT * BL], BF16, tag="XTO")
            XN0 = big.tile([128, NT * 128], BF16, tag="XN0")
            XN1 = big.tile([128, NT * 128], BF16, tag="XN1")
            Gb = big.tile([128, NT * CO], BF16, tag="Gb")
            T1 = big.tile([128, NT * C * 8], BF16, tag="T1")
            T2 = big.tile([128, NT * C * 4], BF16, tag="T2")
            T3 = big.tile([128, NT * C * 2], BF16, tag="T3")
            Q = big.tile([128, NT * C], F32, tag="Q")

            b_sb = small.tile([128, RT * C], F32, tag="b")
            expb = small.tile([128, RT * C], F32, tag="expb")
            esum = small.tile([128, RT * C], F32, tag="esum")
            c_sb = small.tile([128, RT * C], F32, tag="c")
            c16 = small.tile([128, RT * C], BF16, tag="c16")
            crep = small.tile([128, RT * C * O], BF16, tag="crep")
            abar = small.tile([128, RT * C], F32, tag="abar")
            zp = small.tile([128, C], F32, tag="zp")
            zr = small.tile([128, C], F32, tag="zr")
            # squash working set, full batch: [128, (h, co)] = [128, 320]
            se = small.tile([128, 2 * CO], F32, tag="se")
            ab = small.tile([128, 2 * CO], F32, tag="ab")
            sq = small.tile([128, 2 * CO], F32, tag="sq")
            rd = small.tile([128, 2 * CO], F32, tag="rd")
            num = small.tile([128, 2 * CO], F32, tag="num")
            vv = small.tile([128, 2 * CO], F32, tag="vv")
            vbf = small.tile([128, 2 * CO], BF16, tag="vbf")

            for _rep in range(repeat):
                # --- input DMAs, interleaved by first use (DMA dev serializes)
                NWC = NT // 4          # Wg chunk: 18 tiles
                NNC = NT // 4          # XN chunk: 18 tiles
                for ch in range(4):
                    nc.sync.dma_start(
                        out=Wg[:, ch * NWC * CO:(ch + 1) * NWC * CO],
                        in_=wg_d[:, ch * NWC * CO:(ch + 1) * NWC * CO])
                    nc.sync.dma_start(
                        out=XTF[:, ch * NWC * B:(ch + 1) * NWC * B],
                        in_=xtf_d[:, ch * NWC * B:(ch + 1) * NWC * B])
                for ch in range(4):
                    sl = slice(ch * NNC * 128, (ch + 1) * NNC * 128)
                    nc.sync.dma_start(out=XN0[:, sl], in_=xn0_d[:, sl])
                    nc.sync.dma_start(out=XN1[:, sl], in_=xn1_d[:, sl])
                nc.sync.dma_start(out=XTO[:, :], in_=xto_d[:, :])
                nc.vector.memset(b_sb[:, :], 0.0)

                # --- persistent PSUM tiles (one bank per accumulation chain)
                sh0 = sps.tile([128, CO], F32, tag="sh0")
                sh1 = sps.tile([128, CO], F32, tag="sh1")
                s_h = [sh0, sh1]
                warm_ps = sps.tile([128, 4], F32, tag="warm")

                def warm(src, p=128):
                    """Keep the PE p-state ramped through engine-idle windows:
                    a 4-row dummy matmul whose moving operand is the output of
                    the op that gates the next real PE work."""
                    if src.dtype != BF16:
                        src = src.bitcast(BF16)
                    nc.tensor.matmul(warm_ps[0:1, 0:4], Wg[0:p, 0:1],
                                     src[0:p, 0:4], start=True, stop=True)

                for k in range(NITER):
                    if k > 0:
                        # c = softmax over r (partitions x RT tiles)
                        nc.scalar.activation(expb[:, :], b_sb[:, :], ACT.Exp)
                        warm(expb[:, :])
                        nc.gpsimd.partition_all_reduce(
                            esum[:, :], expb[:, :], channels=128,
                            reduce_op=bass_isa.ReduceOp.add)
                        warm(esum[:, :])
                        nc.vector.tensor_reduce(
                            zp[:, :],
                            esum[:, :].rearrange("p (T c) -> p c T", c=C),
                            axis=mybir.AxisListType.X, op=ALU.add)
                        warm(zp[:, :])
                        nc.vector.reciprocal(zr[:, :], zp[:, :])
                        nc.vector.tensor_tensor(
                            c_sb[:, :].rearrange("p (T c) -> p T c", c=C),
                            expb[:, :].rearrange("p (T c) -> p T c", c=C),
                            zr[:, :].unsqueeze(1).broadcast_to((128, RT, C)),
                            op=ALU.mult)
                        warm(c_sb[:, :])
                        nc.scalar.activation(c16[:, :], c_sb[:, :], ACT.Copy)
                        # crep[p,(T,c,o)] = c16[p,(T,c)] replicated over o
                        nc.vector.tensor_copy(
                            crep[:, :].rearrange("p (T c o) -> p T c o",
                                                 c=C, o=O),
                            c16[:, :].rearrange("p (T c) -> p T c", c=C)
                            .unsqueeze(3).broadcast_to((128, RT, C, O)))
                        warm(crep[:, :])
                        # W' = Wg * crep, chunked per i so the s-matmul can
                        # stream behind it; i 0-6 on DVE, 7 on Pool (Pool
                        # tensor ops model ~2.5x slower per element).
                        FI = RT * C * O  # 1440, one i-plane
                        for ii in range(I):
                            nc.vector.tensor_tensor(
                                Wp[:, ii * FI:(ii + 1) * FI],
                                Wg[:, ii * FI:(ii + 1) * FI],
                                crep[:, :], op=ALU.mult)
                            if ii == 0:
                                warm(Wp[:, 0:4])

                    mov = Wg if k == 0 else Wp

                    if k < NITER - 1:
                        # s matmul, full batch: out [b-half, co], 1 bank each
                        for u in range(NT):
                            for h in range(2):
                                nc.tensor.matmul(
                                    s_h[h][:, :],
                                    XTF[:, u * B + h * 128:u * B + h * 128 + 128],
                                    mov[:, u * CO:(u + 1) * CO],
                                    start=(u == 0), stop=(u == NT - 1))
                        P, width = 128, 2 * CO
                        sq_src = [(s_h[0][:, :], 0, CO), (s_h[1][:, :], CO, CO)]
                    else:
                        # final pass: own 32 batches only, swapped orientation
                        for u in range(NT):
                            for hh in range(2):
                                nc.tensor.matmul(
                                    s_h[hh][0:80, 0:BL],
                                    mov[:, u * CO + hh * 80:u * CO + hh * 80 + 80],
                                    XTO[:, u * BL:(u + 1) * BL],
                                    start=(u == 0), stop=(u == NT - 1))
                        P, width = 80, 2 * BL
                        sq_src = [(s_h[0][0:80, 0:BL], 0, BL),
                                  (s_h[1][0:80, 0:BL], BL, BL)]

                    # squash: v = s*|s| / (1+s^2)
                    for src, off, w in sq_src:
                        nc.scalar.activation(se[0:P, off:off + w], src,
                                             ACT.Copy,
                                             scale=(1.0 / R if k == 0 else 1.0))
                    warm(se[:, :], p=P)
                    nc.scalar.activation(ab[0:P, 0:width], se[0:P, 0:width],
                                         ACT.Abs)
                    warm(ab[:, :], p=P)
                    nc.vector.tensor_mul(sq[0:P, 0:width], se[0:P, 0:width],
                                         se[0:P, 0:width])
                    warm(sq[:, :], p=P)
                    nc.vector.tensor_scalar_add(sq[0:P, 0:width],
                                                sq[0:P, 0:width], 1.0)
                    nc.vector.reciprocal(rd[0:P, 0:width], sq[0:P, 0:width])
                    warm(rd[:, :], p=P)
                    nc.vector.tensor_mul(num[0:P, 0:width], se[0:P, 0:width],
                                         ab[0:P, 0:width])
                    warm(num[:, :], p=P)
                    nc.vector.tensor_mul(vv[0:P, 0:width], num[0:P, 0:width],
                                         rd[0:P, 0:width])
                    warm(vv[:, :], p=P)

                    if k == NITER - 1:
                        nc.sync.dma_start(out=out_d[:, :], in_=vv[0:80, 0:width])
                        continue

                    nc.scalar.activation(vbf[:, :], vv[:, :], ACT.Copy,
                                         scale=1.0 / B)
                    warm(vbf[:, :])

                    # G matmul over b (2 chained halves per region); PSUM
                    # drains to bf16 round-robin ACT/Pool; the agreement
                    # P-mult + o-tree runs per u-quarter so only the last
                    # quarter's tree is exposed past the final G matmul.
                    # 12 big-groups of 6 K-tiles; each lands in a 2-bank PSUM
                    # tile (3 regions per bank, 480+pad layout) and drains in
                    # one strided ACT copy (GPSIMD cannot touch PSUM).
                    NQ = 4
                    for quar in range(NQ):
                        bg0 = quar * (NT // (6 * NQ))
                        for bg in range(bg0, bg0 + NT // (6 * NQ)):
                            g_ps = gps.tile([128, 1024], F32, tag="g")
                            for j in range(6):
                                u = 6 * bg + j
                                col = (j // 3) * 512 + (j % 3) * CO
                                for h, XN in ((0, XN0), (1, XN1)):
                                    nc.tensor.matmul(
                                        g_ps[:, col:col + CO],
                                        XN[:, u * 128:(u + 1) * 128],
                                        vbf[:, h * CO:(h + 1) * CO],
                                        start=(h == 0), stop=(h == 1))
                            nc.scalar.copy(
                                Gb[:, bg * 6 * CO:(bg + 1) * 6 * CO]
                                .rearrange("p (s f) -> p s f", s=2),
                                g_ps[:, :].rearrange("p (s f) -> p s f",
                                                     s=2)[:, :, 0:480])
                        # P = Wg .* Gb for this u-quarter (packed bf16);
                        # first chunk of quarters 0-2 on Pool, rest on DVE
                        QW = NT // NQ * CO  # 2880 cols per quarter
                        qb = quar * QW
                        last = quar == NQ - 1
                        for hch in range(2):
                            sl = slice(qb + hch * 1440, qb + (hch + 1) * 1440)
                            eng = nc.gpsimd if (hch == 0 and not last) \
                                else nc.vector
                            eng.tensor_tensor(Wp[:, sl], Wg[:, sl],
                                              Gb[:, sl], op=ALU.mult)
                            if last:
                                warm(Wp[:, sl])
                        # o-tree for this quarter
                        o2 = QW // 2
                        Pv = Wp[:, qb:qb + QW].rearrange(
                            "p (f o) -> p f o", o=O)
                        T1v = T1[:, quar * o2:quar * o2 + o2].rearrange(
                            "p (f o) -> p f o", o=O // 2)
                        nc.vector.tensor_tensor(
                            T1v, Pv[:, :, 0:8], Pv[:, :, 8:16], op=ALU.add)
                        if last:
                            warm(T1[:, quar * o2:quar * o2 + 4])
                        o4 = QW // 4
                        T2v = T2[:, quar * o4:quar * o4 + o4].rearrange(
                            "p (f o) -> p f o", o=O // 4)
                        nc.vector.tensor_tensor(
                            T2v, T1v[:, :, 0:4], T1v[:, :, 4:8], op=ALU.add)
                        if last:
                            warm(T2[:, quar * o4:quar * o4 + 4])
                        o8 = QW // 8
                        T3v = T3[:, quar * o8:quar * o8 + o8].rearrange(
                            "p (f o) -> p f o", o=O // 8)
                        nc.vector.tensor_tensor(
                            T3v, T2v[:, :, 0:2], T2v[:, :, 2:4], op=ALU.add)
                        if last:
                            warm(T3[:, quar * o8:quar * o8 + 4])
                        o16 = QW // 16
                        nc.vector.tensor_tensor(
                            Q[:, quar * o16:quar * o16 + o16].rearrange(
                                "p (f o) -> p f o", o=1),
                            T3v[:, :, 0:1], T3v[:, :, 1:2], op=ALU.add)
                        if last:
                            warm(Q[:, quar * o16:quar * o16 + 2])

                    # abar[r,c] = sum_i Q[p, (i T c)]: halving adds over the
                    # contiguous i axis (TensorReduce has no fast path)
                    QH = NT * C  # 720
                    nc.vector.tensor_add(Q[:, 0:QH // 2], Q[:, 0:QH // 2],
                                         Q[:, QH // 2:QH])
                    nc.vector.tensor_add(Q[:, 0:QH // 4], Q[:, 0:QH // 4],
                                         Q[:, QH // 4:QH // 2])
                    nc.vector.tensor_add(Q[:, 0:QH // 8], Q[:, 0:QH // 8],
                                         Q[:, QH // 8:QH // 4])
                    warm(Q[:, 0:2])
                    # full-batch mean already in vbf scale; b += abar directly
                    nc.vector.tensor_add(b_sb[:, :], b_sb[:, :],
                                         Q[:, 0:QH // 8])
                    warm(b_sb[:, :])

    nc.compile()
    return nc


def _host_inputs(x, W):
    # rows ordered (i, r), r-major tiles: tile u = i*RT + T holds
    # r = T*128 + p on partition p.  Free order (c, o).  Partition-major.
    wg = np.ascontiguousarray(
        W.transpose(3, 0, 1, 2).reshape(I, RT, 128, CO)
        .transpose(2, 0, 1, 3).reshape(128, NT * CO)).astype(ml_dtypes.bfloat16)
    xtf = np.ascontiguousarray(
        x.transpose(2, 1, 0).reshape(I, RT, 128, B)
        .transpose(2, 0, 1, 3).reshape(128, NT * B)).astype(ml_dtypes.bfloat16)
    xn = [np.ascontiguousarray(
        x[h * 128:(h + 1) * 128].transpose(0, 2, 1).reshape(128, NT * 128))
        .astype(ml_dtypes.bfloat16) for h in range(2)]
    in_maps = []
    for cidx in range(NCORES):
        xc = x[cidx * BL:(cidx + 1) * BL]          # (BL, R, I)
        xto = np.ascontiguousarray(
            xc.transpose(2, 1, 0).reshape(I, RT, 128, BL)
            .transpose(2, 0, 1, 3).reshape(128, NT * BL)).astype(
            ml_dtypes.bfloat16)
        in_maps.append({"wg": wg, "xtf": xtf, "xto": xto,
                        "xn0": xn[0], "xn1": xn[1]})
    return in_maps


def kernel(x, W):
    x = np.ascontiguousarray(np.asarray(x, dtype=np.float32))
    W = np.ascontiguousarray(np.asarray(W, dtype=np.float32))
    assert x.shape == (B, R, I) and W.shape == (R, C, O, I)
    if "nc" not in _CACHE:
        _CACHE["nc"] = _build()
    nc = _CACHE["nc"]
    in_maps = _host_inputs(x, W)
    res = bass_utils.run_bass_kernel_spmd(nc, in_maps,
                                          core_ids=list(range(NCORES)))
    # out [80, 2*BL]: out[p, hh*BL + b] = v[b_own, co = hh*80 + p]
    vs = []
    for r in res.results:
        o = r["out"].reshape(80, 2, BL).transpose(1, 0, 2).reshape(CO, BL)
        vs.append(o.T.reshape(BL, C, O))
    return np.concatenate(vs, axis=0)[..., None].astype(np.float32)


# revision 3
# speedup vs baseline: 1.0228x; 1.0092x over previous
"""DigitCaps (CapsNet dynamic routing) kernel for 8 Trainium2 NeuronCores.

Reference math:
  u_hat[b,r,c,o] = sum_i W[r,c,o,i] * x[b,r,i]
  b_ij = 0;  3 routing iterations:
     c = softmax_r(b);  s[b,c,o] = sum_r c[r,c] u_hat[b,r,c,o];
     v = squash(s);     b += mean_b(sum_o u_hat[b,r,c,o] v[b,c,o])
  returns v[..., None]  (256, 10, 16, 1)

Strategy: ZERO-COMMUNICATION full replication.  The routing logits b_ij are
batch-shared, so the routing trajectory is identical on every core; each
core computes it for the FULL batch (B=256) for iterations 0-1 (whose only
product is the shared b_ij update), then computes the final-iteration
capsule outputs only for its OWN 32-batch slice (per-core xto input).  No
collectives, no remote DMA, no cross-core sync of any kind.

u_hat (189 MB) is never materialized: the routing coefficients are folded
into the weights so every pass is a dense matmul over K=(i,r)=9216:
    s-matmul:  s[b,(c,o)]   = sum_K  XTF[K,b] * (c-scaled Wg)[K,(c,o)]
    G-matmul:  G[K,(c,o)]   = sum_b  XN[b,K] * (v[b,(c,o)]/B)
    agreement: abar[r,c]    = sum_{i,o} Wg .* G
Rows are ordered (i, r) with r-major tiles of 128, so partition p of tile
u=(i,T) holds r = T*128+p.  x is DMA'd in both layouts (K-major XTF for
the s-matmul, b-major XN halves for the G-matmul), chunk-interleaved in
first-use order so every pass streams behind the serial DMA device.  The
o-reduction of abar is a tree of packed bf16 adds run per u-half so it
pipelines into the G drains, the i-reduction a free-axis DVE reduce, the
softmax a partition_all_reduce + free-axis reduce, and the c-fold a packed
bf16 broadcast multiply (DVE-heavy split; Pool tensor ops model ~2.5x
slower per element).  PSUM accumulation chains each own a full bank (2 KB
zero region).  Matmuls run in bf16; softmax and squash in fp32.
"""
import sys
if '/opt/trn_rl_repo' not in sys.path:
    sys.path.insert(0, '/opt/trn_rl_repo')
import numpy as np
import ml_dtypes

import concourse.bass as bass
import concourse.bacc as bacc
import concourse.mybir as mybir
import concourse.tile as tile
from concourse import bass_utils
from concourse import bass_isa

BF16 = mybir.dt.bfloat16
F32 = mybir.dt.float32
ALU = mybir.AluOpType
ACT = mybir.ActivationFunctionType

B, R, C, O, I = 256, 1152, 10, 16, 8
NCORES = 8
BL = B // NCORES          # 32 own batch (final pass only)
RT = 9                    # r tiles of 128 (per i)
NT = 72                   # (i, r) tiles of 128: u = i*RT + T
CO = C * O                # 160, free order (c,o): idx = c*O + o
NITER = 3

_CACHE = {}


def _build(n_cores=NCORES, repeat=1):
    nc = bacc.Bacc("TRN2", target_bir_lowering=False, debug=False,
                   num_devices=n_cores)
    wg_d = nc.dram_tensor("wg", [128, NT * CO], BF16, kind="ExternalInput")
    xtf_d = nc.dram_tensor("xtf", [128, NT * B], BF16, kind="ExternalInput")
    xto_d = nc.dram_tensor("xto", [128, NT * BL], BF16, kind="ExternalInput")
    xn0_d = nc.dram_tensor("xn0", [128, NT * 128], BF16, kind="ExternalInput")
    xn1_d = nc.dram_tensor("xn1", [128, NT * 128], BF16, kind="ExternalInput")
    out_d = nc.dram_tensor("out", [80, 2 * BL], F32, kind="ExternalOutput")

    with tile.TileContext(nc) as tc:
        with (
            tc.tile_pool(name="big", bufs=1) as big,
            tc.tile_pool(name="small", bufs=1) as small,
            tc.tile_pool(name="sps", bufs=1, space="PSUM") as sps,
            tc.tile_pool(name="gps", bufs=2, space="PSUM") as gps,
        ):
            Wg = big.tile([128, NT * CO], BF16, tag="Wg")
            Wp = big.tile([128, NT * CO], BF16, tag="Wp")
            XTF = big.tile([128, NT * B], BF16, tag="XTF")
            XTO = big.tile([128, NT * BL], BF16, tag="XTO")
            XN0 = big.tile([128, NT * 128], BF16, tag="XN0")
            XN1 = big.tile([128, NT * 128], BF16, tag="XN1")
            Gb = big.tile([128, NT * CO], BF16, tag="Gb")
            T1 = big.tile([128, NT * C * 8], BF16, tag="T1")
            T2 = big.tile([128, NT * C * 4], BF16, tag="T2")
            T3 = big.tile([128, NT * C * 2], BF16, tag="T3")
            Q = big.tile([128, NT * C], F32, tag="Q")

            b_sb = small.tile([128, RT * C], F32, tag="b")
            expb = small.tile([128, RT * C], F32, tag="expb")
            esum = small.tile([128, RT * C], F32, tag="esum")
            c_sb = small.tile([128, RT * C], F32, tag="c")
            c16 = small.tile([128, RT * C], BF16, tag="c16")
            crep = small.tile([128, RT * C * O], BF16, tag="crep")
            abar = small.tile([128, RT * C], F32, tag="abar")
            zp = small.tile([128, C], F32, tag="zp")
            zr = small.tile([128, C], F32, tag="zr")
            # squash working set, full batch: [128, (h, co)] = [128, 320]
            se = small.tile([128, 2 * CO], F32, tag="se")
            ab = small.tile([128, 2 * CO], F32, tag="ab")
            sq = small.tile([128, 2 * CO], F32, tag="sq")
            rd = small.tile([128, 2 * CO], F32, tag="rd")
            num = small.tile([128, 2 * CO], F32, tag="num")
            vv = small.tile([128, 2 * CO], F32, tag="vv")
            vbf = small.tile([128, 2 * CO], BF16, tag="vbf")

            for _rep in range(repeat):
                # --- input DMAs, interleaved by first use (DMA dev serializes)
                NWC = NT // 4          # Wg chunk: 18 tiles
                NNC = NT // 4          # XN chunk: 18 tiles
                for ch in range(4):
                    nc.sync.dma_start(
                        out=Wg[:, ch * NWC * CO:(ch + 1) * NWC * CO],
                        in_=wg_d[:, ch * NWC * CO:(ch + 1) * NWC * CO])
                    nc.sync.dma_start(
                        out=XTF[:, ch * NWC * B:(ch + 1) * NWC * B],
                        in_=xtf_d[:, ch * NWC * B:(ch + 1) * NWC * B])
                for ch in range(4):
                    sl = slice(ch * NNC * 128, (ch + 1) * NNC * 128)
                    nc.sync.dma_start(out=XN0[:, sl], in_=xn0_d[:, sl])
                    nc.sync.dma_start(out=XN1[:, sl], in_=xn1_d[:, sl])
                nc.sync.dma_start(out=XTO[:, :], in_=xto_d[:, :])
                nc.vector.memset(b_sb[:, :], 0.0)

                # --- persistent PSUM tiles (one bank per accumulation chain)
                sh0 = sps.tile([128, CO], F32, tag="sh0")
                sh1 = sps.tile([128, CO], F32, tag="sh1")
                s_h = [sh0, sh1]
                warm_ps = sps.tile([128, 4], F32, tag="warm")

                def warm(src, p=128):
                    """Keep the PE p-state ramped through engine-idle windows:
                    a 4-row dummy matmul whose moving operand is the output of
                    the op that gates the next real PE work."""
                    if src.dtype != BF16:
                        src = src.bitcast(BF16)
                    nc.tensor.matmul(warm_ps[0:1, 0:4], Wg[0:p, 0:1],
                                     src[0:p, 0:4], start=True, stop=True)

                for k in range(NITER):
                    if k > 0:
                        # c = softmax over r (partitions x RT tiles)
                        nc.scalar.activation(expb[:, :], b_sb[:, :], ACT.Exp)
                        warm(expb[:, :])
                        nc.gpsimd.partition_all_reduce(
                            esum[:, :], expb[:, :], channels=128,
                            reduce_op=bass_isa.ReduceOp.add)
                        warm(esum[:, :])
                        nc.vector.tensor_reduce(
                            zp[:, :],
                            esum[:, :].rearrange("p (T c) -> p c T", c=C),
                            axis=mybir.AxisListType.X, op=ALU.add)
                        warm(zp[:, :])
                        nc.vector.reciprocal(zr[:, :], zp[:, :])
                        nc.vector.tensor_tensor(
                            c_sb[:, :].rearrange("p (T c) -> p T c", c=C),
                            expb[:, :].rearrange("p (T c) -> p T c", c=C),
                            zr[:, :].unsqueeze(1).broadcast_to((128, RT, C)),
                            op=ALU.mult)
                        warm(c_sb[:, :])
                        nc.vector.tensor_copy(c16[:, :], c_sb[:, :])
                        # crep[p,(T,c,o)] = c16[p,(T,c)] replicated over o
                        nc.vector.tensor_copy(
                            crep[:, :].rearrange("p (T c o) -> p T c o",
                                                 c=C, o=O),
                            c16[:, :].rearrange("p (T c) -> p T c", c=C)
                            .unsqueeze(3).broadcast_to((128, RT, C, O)))
                        warm(crep[:, :])
                        # W' = Wg * crep, chunked per i so the s-matmul can
                        # stream behind it; i 0-6 on DVE, 7 on Pool (Pool
                        # tensor ops model ~2.5x slower per element).
                        FI = RT * C * O  # 1440, one i-plane
                        for ii in range(I):
                            eng = nc.gpsimd if ii == 5 else nc.vector
                            eng.tensor_tensor(
                                Wp[:, ii * FI:(ii + 1) * FI],
                                Wg[:, ii * FI:(ii + 1) * FI],
                                crep[:, :], op=ALU.mult)
                            if ii == 0:
                                warm(Wp[:, 0:4])

                    mov = Wg if k == 0 else Wp

                    if k < NITER - 1:
                        # s matmul, full batch: out [b-half, co], 1 bank each
                        for u in range(NT):
                            for h in range(2):
                                nc.tensor.matmul(
                                    s_h[h][:, :],
                                    XTF[:, u * B + h * 128:u * B + h * 128 + 128],
                                    mov[:, u * CO:(u + 1) * CO],
                                    start=(u == 0), stop=(u == NT - 1))
                        P, width = 128, 2 * CO
                        sq_src = [(s_h[0][:, :], 0, CO), (s_h[1][:, :], CO, CO)]
                    else:
                        # final pass: own 32 batches only, swapped orientation
                        for u in range(NT):
                            for hh in range(2):
                                nc.tensor.matmul(
                                    s_h[hh][0:80, 0:BL],
                                    mov[:, u * CO + hh * 80:u * CO + hh * 80 + 80],
                                    XTO[:, u * BL:(u + 1) * BL],
                                    start=(u == 0), stop=(u == NT - 1))
                        P, width = 80, 2 * BL
                        sq_src = [(s_h[0][0:80, 0:BL], 0, BL),
                                  (s_h[1][0:80, 0:BL], BL, BL)]

                    # squash: v = s*|s| / (1+s^2)
                    for src, off, w in sq_src:
                        nc.scalar.activation(se[0:P, off:off + w], src,
                                             ACT.Copy,
                                             scale=(1.0 / R if k == 0 else 1.0))
                    warm(se[:, :], p=P)
                    nc.scalar.activation(ab[0:P, 0:width], se[0:P, 0:width],
                                         ACT.Abs)
                    warm(ab[:, :], p=P)
                    nc.vector.tensor_mul(sq[0:P, 0:width], se[0:P, 0:width],
                                         se[0:P, 0:width])
                    warm(sq[:, :], p=P)
                    nc.vector.tensor_scalar_add(sq[0:P, 0:width],
                                                sq[0:P, 0:width], 1.0)
                    nc.vector.reciprocal(rd[0:P, 0:width], sq[0:P, 0:width])
                    warm(rd[:, :], p=P)
                    nc.vector.tensor_mul(num[0:P, 0:width], se[0:P, 0:width],
                                         ab[0:P, 0:width])
                    warm(num[:, :], p=P)
                    nc.vector.tensor_mul(vv[0:P, 0:width], num[0:P, 0:width],
                                         rd[0:P, 0:width])
                    warm(vv[:, :], p=P)

                    if k == NITER - 1:
                        nc.sync.dma_start(out=out_d[:, :], in_=vv[0:80, 0:width])
                        continue

                    nc.scalar.activation(vbf[:, :], vv[:, :], ACT.Copy,
                                         scale=1.0 / B)
                    warm(vbf[:, :])

                    # G matmul over b (2 chained halves per region); PSUM
                    # drains to bf16 round-robin ACT/Pool; the agreement
                    # P-mult + o-tree runs per u-quarter so only the last
                    # quarter's tree is exposed past the final G matmul.
                    # 12 big-groups of 6 K-tiles; each lands in a 2-bank PSUM
                    # tile (3 regions per bank, 480+pad layout) and drains in
                    # one strided ACT copy (GPSIMD cannot touch PSUM).
                    NQ = 4
                    for quar in range(NQ):
                        bg0 = quar * (NT // (6 * NQ))
                        for bg in range(bg0, bg0 + NT // (6 * NQ)):
                            g_ps = gps.tile([128, 1024], F32, tag="g")
                            for j in range(6):
                                u = 6 * bg + j
                                col = (j // 3) * 512 + (j % 3) * CO
                                for h, XN in ((0, XN0), (1, XN1)):
                                    nc.tensor.matmul(
                                        g_ps[:, col:col + CO],
                                        XN[:, u * 128:(u + 1) * 128],
                                        vbf[:, h * CO:(h + 1) * CO],
                                        start=(h == 0), stop=(h == 1))
                            nc.scalar.copy(
                                Gb[:, bg * 6 * CO:(bg + 1) * 6 * CO]
                                .rearrange("p (s f) -> p s f", s=2),
                                g_ps[:, :].rearrange("p (s f) -> p s f",
                                                     s=2)[:, :, 0:480])
                        # P = Wg .* Gb for this u-quarter (packed bf16);
                        # first chunk of quarters 0-2 on Pool, rest on DVE
                        QW = NT // NQ * CO  # 2880 cols per quarter
                        qb = quar * QW
                        last = quar == NQ - 1
                        for hch in range(2):
                            sl = slice(qb + hch * 1440, qb + (hch + 1) * 1440)
                            eng = nc.gpsimd if (hch == 0 and not last) \
                                else nc.vector
                            eng.tensor_tensor(Wp[:, sl], Wg[:, sl],
                                              Gb[:, sl], op=ALU.mult)
                            if last:
                                warm(Wp[:, sl])
                        # o-tree for this quarter
                        o2 = QW // 2
                        Pv = Wp[:, qb:qb + QW].rearrange(
                            "p (f o) -> p f o", o=O)
                        T1v = T1[:, quar * o2:quar * o2 + o2].rearrange(
                            "p (f o) -> p f o", o=O // 2)
                        nc.vector.tensor_tensor(
                            T1v, Pv[:, :, 0:8], Pv[:, :, 8:16], op=ALU.add)
                        if last:
                            warm(T1[:, quar * o2:quar * o2 + 4])
                        o4 = QW // 4
                        T2v = T2[:, quar * o4:quar * o4 + o4].rearrange(
                            "p (f o) -> p f o", o=O // 4)
                        nc.vector.tensor_tensor(
                            T2v, T1v[:, :, 0:4], T1v[:, :, 4:8], op=ALU.add)
                        if last:
                            warm(T2[:, quar * o4:quar * o4 + 4])
                        o8 = QW // 8
                        T3v = T3[:, quar * o8:quar * o8 + o8].rearrange(
                            "p (f o) -> p f o", o=O // 8)
                        nc.vector.tensor_tensor(
                            T3v, T2v[:, :, 0:2], T2v[:, :, 2:4], op=ALU.add)
                        if last:
                            warm(T3[:, quar * o8:quar * o8 + 4])
                        o16 = QW // 16
                        nc.vector.tensor_tensor(
                            Q[:, quar * o16:quar * o16 + o16].rearrange(
                                "p (f o) -> p f o", o=1),
                            T3v[:, :, 0:1], T3v[:, :, 1:2], op=ALU.add)
                        if last:
                            warm(Q[:, quar * o16:quar * o16 + 2])

                    # abar[r,c] = sum_i Q[p, (i T c)]: halving adds over the
                    # contiguous i axis (TensorReduce has no fast path)
                    QH = NT * C  # 720
                    nc.vector.tensor_add(Q[:, 0:QH // 2], Q[:, 0:QH // 2],
                                         Q[:, QH // 2:QH])
                    nc.vector.tensor_add(Q[:, 0:QH // 4], Q[:, 0:QH // 4],
                                         Q[:, QH // 4:QH // 2])
                    nc.vector.tensor_add(Q[:, 0:QH // 8], Q[:, 0:QH // 8],
                                         Q[:, QH // 8:QH // 4])
                    warm(Q[:, 0:2])
                    # full-batch mean already in vbf scale; b += abar directly
                    nc.vector.tensor_add(b_sb[:, :], b_sb[:, :],
                                         Q[:, 0:QH // 8])
                    warm(b_sb[:, :])

    nc.compile()
    return nc


def _host_inputs(x, W):
    # rows ordered (i, r), r-major tiles: tile u = i*RT + T holds
    # r = T*128 + p on partition p.  Free order (c, o).  Partition-major.
    wg = np.ascontiguousarray(
        W.transpose(3, 0, 1, 2).reshape(I, RT, 128, CO)
        .transpose(2, 0, 1, 3).reshape(128, NT * CO)).astype(ml_dtypes.bfloat16)
    xtf = np.ascontiguousarray(
        x.transpose(2, 1, 0).reshape(I, RT, 128, B)
        .transpose(2, 0, 1, 3).reshape(128, NT * B)).astype(ml_dtypes.bfloat16)
    xn = [np.ascontiguousarray(
        x[h * 128:(h + 1) * 128].transpose(0, 2, 1).reshape(128, NT * 128))
        .astype(ml_dtypes.bfloat16) for h in range(2)]
    in_maps = []
    for cidx in range(NCORES):
        xc = x[cidx * BL:(cidx + 1) * BL]          # (BL, R, I)
        xto = np.ascontiguousarray(
            xc.transpose(2, 1, 0).reshape(I, RT, 128, BL)
            .transpose(2, 0, 1, 3).reshape(128, NT * BL)).astype(
            ml_dtypes.bfloat16)
        in_maps.append({"wg": wg, "xtf": xtf, "xto": xto,
                        "xn0": xn[0], "xn1": xn[1]})
    return in_maps


def kernel(x, W):
    x = np.ascontiguousarray(np.asarray(x, dtype=np.float32))
    W = np.ascontiguousarray(np.asarray(W, dtype=np.float32))
    assert x.shape == (B, R, I) and W.shape == (R, C, O, I)
    if "nc" not in _CACHE:
        _CACHE["nc"] = _build()
    nc = _CACHE["nc"]
    in_maps = _host_inputs(x, W)
    res = bass_utils.run_bass_kernel_spmd(nc, in_maps,
                                          core_ids=list(range(NCORES)))
    # out [80, 2*BL]: out[p, hh*BL + b] = v[b_own, co = hh*80 + p]
    vs = []
    for r in res.results:
        o = r["out"].reshape(80, 2, BL).transpose(1, 0, 2).reshape(CO, BL)
        vs.append(o.T.reshape(BL, C, O))
    return np.concatenate(vs, axis=0)[..., None].astype(np.float32)


# revision 4
# speedup vs baseline: 1.0309x; 1.0079x over previous
"""DigitCaps (CapsNet dynamic routing) kernel for 8 Trainium2 NeuronCores.

Reference math:
  u_hat[b,r,c,o] = sum_i W[r,c,o,i] * x[b,r,i]
  b_ij = 0;  3 routing iterations:
     c = softmax_r(b);  s[b,c,o] = sum_r c[r,c] u_hat[b,r,c,o];
     v = squash(s);     b += mean_b(sum_o u_hat[b,r,c,o] v[b,c,o])
  returns v[..., None]  (256, 10, 16, 1)

Strategy: ZERO-COMMUNICATION full replication.  The routing logits b_ij are
batch-shared, so the routing trajectory is identical on every core; each
core computes it for the FULL batch (B=256) for iterations 0-1 (whose only
product is the shared b_ij update), then computes the final-iteration
capsule outputs only for its OWN 32-batch slice (per-core xto input).  No
collectives, no remote DMA, no cross-core sync of any kind.

u_hat (189 MB) is never materialized: the routing coefficients are folded
into the weights so every pass is a dense matmul over K=(i,r)=9216:
    s-matmul:  s[b,(c,o)]   = sum_K  XTF[K,b] * (c-scaled Wg)[K,(c,o)]
    G-matmul:  G[K,(c,o)]   = sum_b  XN[b,K] * (v[b,(c,o)]/B)
    agreement: abar[r,c]    = sum_{i,o} Wg .* G
Rows are ordered (i, r) with r-major tiles of 128, so partition p of tile
u=(i,T) holds r = T*128+p.  x is DMA'd in both layouts (K-major XTF for
the s-matmul, b-major XN halves for the G-matmul), chunk-interleaved in
first-use order so every pass streams behind the serial DMA device.  The
o-reduction of abar is a tree of packed bf16 adds run per u-quarter so it
pipelines into the G drains, the i-reduction paired adds per quarter, the
softmax a partition_all_reduce + free-axis reduce, and the c-fold a packed
bf16 broadcast multiply (DVE-heavy split; Pool tensor ops model ~2.5x
slower per element).  PSUM accumulation chains each own a full bank (2 KB
zero region).  Matmuls run in bf16; softmax and squash in fp32.
"""
import sys
if '/opt/trn_rl_repo' not in sys.path:
    sys.path.insert(0, '/opt/trn_rl_repo')
import numpy as np
import ml_dtypes

import concourse.bass as bass
import concourse.bacc as bacc
import concourse.mybir as mybir
import concourse.tile as tile
from concourse import bass_utils
from concourse import bass_isa

BF16 = mybir.dt.bfloat16
F32 = mybir.dt.float32
ALU = mybir.AluOpType
ACT = mybir.ActivationFunctionType

B, R, C, O, I = 256, 1152, 10, 16, 8
NCORES = 8
BL = B // NCORES          # 32 own batch (final pass only)
RT = 9                    # r tiles of 128 (per i)
NT = 72                   # (i, r) tiles of 128: u = i*RT + T
CO = C * O                # 160, free order (c,o): idx = c*O + o
NITER = 3

_CACHE = {}


def _build(n_cores=NCORES, repeat=1):
    nc = bacc.Bacc("TRN2", target_bir_lowering=False, debug=False,
                   num_devices=n_cores)
    wg_d = nc.dram_tensor("wg", [128, NT * CO], BF16, kind="ExternalInput")
    xtf_d = nc.dram_tensor("xtf", [128, NT * B], BF16, kind="ExternalInput")
    xto_d = nc.dram_tensor("xto", [128, NT * BL], BF16, kind="ExternalInput")
    xn0_d = nc.dram_tensor("xn0", [128, NT * 128], BF16, kind="ExternalInput")
    xn1_d = nc.dram_tensor("xn1", [128, NT * 128], BF16, kind="ExternalInput")
    out_d = nc.dram_tensor("out", [80, 2 * BL], F32, kind="ExternalOutput")

    with tile.TileContext(nc) as tc:
        with (
            tc.tile_pool(name="big", bufs=1) as big,
            tc.tile_pool(name="small", bufs=1) as small,
            tc.tile_pool(name="sps", bufs=1, space="PSUM") as sps,
            tc.tile_pool(name="gps", bufs=2, space="PSUM") as gps,
        ):
            Wg = big.tile([128, NT * CO], BF16, tag="Wg")
            Wp = big.tile([128, NT * CO], BF16, tag="Wp")
            XTF = big.tile([128, NT * B], BF16, tag="XTF")
            XTO = big.tile([128, NT * BL], BF16, tag="XTO")
            XN0 = big.tile([128, NT * 128], BF16, tag="XN0")
            XN1 = big.tile([128, NT * 128], BF16, tag="XN1")
            Gb = big.tile([128, NT * CO], BF16, tag="Gb")
            T1 = big.tile([128, NT * C * 8], BF16, tag="T1")
            T2 = big.tile([128, NT * C * 4], BF16, tag="T2")
            T3 = big.tile([128, NT * C * 2], BF16, tag="T3")
            Q = big.tile([128, NT * C], F32, tag="Q")

            b_sb = small.tile([128, RT * C], F32, tag="b")
            expb = small.tile([128, RT * C], F32, tag="expb")
            esum = small.tile([128, RT * C], F32, tag="esum")
            c_sb = small.tile([128, RT * C], F32, tag="c")
            c16 = small.tile([128, RT * C], BF16, tag="c16")
            crep = small.tile([128, RT * C * O], BF16, tag="crep")
            zp = small.tile([128, C], F32, tag="zp")
            zr = small.tile([128, C], F32, tag="zr")
            # squash working set, full batch: [128, (h, co)] = [128, 320]
            se = small.tile([128, 2 * CO], F32, tag="se")
            ab = small.tile([128, 2 * CO], F32, tag="ab")
            sq = small.tile([128, 2 * CO], F32, tag="sq")
            rd = small.tile([128, 2 * CO], F32, tag="rd")
            num = small.tile([128, 2 * CO], F32, tag="num")
            vv = small.tile([128, 2 * CO], F32, tag="vv")
            vbf = small.tile([128, 2 * CO], BF16, tag="vbf")

            for _rep in range(repeat):
                # --- input DMAs, interleaved by first use (DMA dev serializes)
                NWC = NT // 4          # Wg chunk: 18 tiles
                NNC = NT // 4          # XN chunk: 18 tiles
                for ch in range(4):
                    nc.sync.dma_start(
                        out=Wg[:, ch * NWC * CO:(ch + 1) * NWC * CO],
                        in_=wg_d[:, ch * NWC * CO:(ch + 1) * NWC * CO])
                    nc.sync.dma_start(
                        out=XTF[:, ch * NWC * B:(ch + 1) * NWC * B],
                        in_=xtf_d[:, ch * NWC * B:(ch + 1) * NWC * B])
                for ch in range(4):
                    sl = slice(ch * NNC * 128, (ch + 1) * NNC * 128)
                    nc.sync.dma_start(out=XN0[:, sl], in_=xn0_d[:, sl])
                    nc.sync.dma_start(out=XN1[:, sl], in_=xn1_d[:, sl])
                nc.sync.dma_start(out=XTO[:, :], in_=xto_d[:, :])
                nc.vector.memset(b_sb[:, :], 0.0)

                # --- persistent PSUM tiles (one bank per accumulation chain)
                sh0 = sps.tile([128, CO], F32, tag="sh0")
                sh1 = sps.tile([128, CO], F32, tag="sh1")
                s_h = [sh0, sh1]
                warm_ps = sps.tile([128, 4], F32, tag="warm")

                def warm(src, p=128):
                    """Keep the PE p-state ramped through engine-idle windows:
                    a 4-row dummy matmul whose moving operand is the output of
                    the op that gates the next real PE work."""
                    if src.dtype != BF16:
                        src = src.bitcast(BF16)
                    nc.tensor.matmul(warm_ps[0:1, 0:4], Wg[0:p, 0:1],
                                     src[0:p, 0:4], start=True, stop=True)

                for k in range(NITER):
                    if k > 0:
                        # c = softmax over r (partitions x RT tiles)
                        nc.scalar.activation(expb[:, :], b_sb[:, :], ACT.Exp)
                        warm(expb[:, :])
                        nc.gpsimd.partition_all_reduce(
                            esum[:, :], expb[:, :], channels=128,
                            reduce_op=bass_isa.ReduceOp.add)
                        warm(esum[:, :])
                        nc.vector.tensor_reduce(
                            zp[:, :],
                            esum[:, :].rearrange("p (T c) -> p c T", c=C),
                            axis=mybir.AxisListType.X, op=ALU.add)
                        warm(zp[:, :])
                        nc.vector.reciprocal(zr[:, :], zp[:, :])
                        nc.vector.tensor_tensor(
                            c_sb[:, :].rearrange("p (T c) -> p T c", c=C),
                            expb[:, :].rearrange("p (T c) -> p T c", c=C),
                            zr[:, :].unsqueeze(1).broadcast_to((128, RT, C)),
                            op=ALU.mult)
                        warm(c_sb[:, :])
                        nc.vector.tensor_copy(c16[:, :], c_sb[:, :])
                        # crep[p,(T,c,o)] = c16[p,(T,c)] replicated over o
                        nc.vector.tensor_copy(
                            crep[:, :].rearrange("p (T c o) -> p T c o",
                                                 c=C, o=O),
                            c16[:, :].rearrange("p (T c) -> p T c", c=C)
                            .unsqueeze(3).broadcast_to((128, RT, C, O)))
                        warm(crep[:, :])
                        # W' = Wg * crep, chunked per i so the s-matmul can
                        # stream behind it; i 0-6 on DVE, 7 on Pool (Pool
                        # tensor ops model ~2.5x slower per element).
                        FI = RT * C * O  # 1440, one i-plane
                        for ii in range(I):
                            eng = nc.gpsimd if ii == 5 else nc.vector
                            eng.tensor_tensor(
                                Wp[:, ii * FI:(ii + 1) * FI],
                                Wg[:, ii * FI:(ii + 1) * FI],
                                crep[:, :], op=ALU.mult)
                            if ii == 0:
                                warm(Wp[:, 0:4])

                    mov = Wg if k == 0 else Wp

                    if k < NITER - 1:
                        # s matmul, full batch: out [b-half, co], 1 bank each
                        for u in range(NT):
                            for h in range(2):
                                nc.tensor.matmul(
                                    s_h[h][:, :],
                                    XTF[:, u * B + h * 128:u * B + h * 128 + 128],
                                    mov[:, u * CO:(u + 1) * CO],
                                    start=(u == 0), stop=(u == NT - 1))
                        P, width = 128, 2 * CO
                        sq_src = [(s_h[0][:, :], 0, CO), (s_h[1][:, :], CO, CO)]
                    else:
                        # final pass: own 32 batches only, swapped orientation
                        for u in range(NT):
                            for hh in range(2):
                                nc.tensor.matmul(
                                    s_h[hh][0:80, 0:BL],
                                    mov[:, u * CO + hh * 80:u * CO + hh * 80 + 80],
                                    XTO[:, u * BL:(u + 1) * BL],
                                    start=(u == 0), stop=(u == NT - 1))
                        P, width = 80, 2 * BL
                        sq_src = [(s_h[0][0:80, 0:BL], 0, BL),
                                  (s_h[1][0:80, 0:BL], BL, BL)]

                    # squash: v = s*|s| / (1+s^2)
                    for src, off, w in sq_src:
                        nc.scalar.activation(se[0:P, off:off + w], src,
                                             ACT.Copy,
                                             scale=(1.0 / R if k == 0 else 1.0))
                    warm(se[:, :], p=P)
                    nc.scalar.activation(ab[0:P, 0:width], se[0:P, 0:width],
                                         ACT.Abs)
                    warm(ab[:, :], p=P)
                    nc.vector.tensor_mul(sq[0:P, 0:width], se[0:P, 0:width],
                                         se[0:P, 0:width])
                    warm(sq[:, :], p=P)
                    nc.vector.tensor_scalar_add(sq[0:P, 0:width],
                                                sq[0:P, 0:width], 1.0)
                    nc.vector.reciprocal(rd[0:P, 0:width], sq[0:P, 0:width])
                    warm(rd[:, :], p=P)
                    nc.vector.tensor_mul(num[0:P, 0:width], se[0:P, 0:width],
                                         ab[0:P, 0:width])
                    warm(num[:, :], p=P)
                    nc.vector.tensor_mul(vv[0:P, 0:width], num[0:P, 0:width],
                                         rd[0:P, 0:width])
                    warm(vv[:, :], p=P)

                    if k == NITER - 1:
                        nc.sync.dma_start(out=out_d[:, :], in_=vv[0:80, 0:width])
                        continue

                    nc.scalar.activation(vbf[:, :], vv[:, :], ACT.Copy,
                                         scale=1.0 / B)
                    warm(vbf[:, :])

                    # G matmul over b (2 chained halves per region); PSUM
                    # drains to bf16 round-robin ACT/Pool; the agreement
                    # P-mult + o-tree runs per u-quarter so only the last
                    # quarter's tree is exposed past the final G matmul.
                    # 12 big-groups of 6 K-tiles; each lands in a 2-bank PSUM
                    # tile (3 regions per bank, 480+pad layout) and drains in
                    # one strided ACT copy (GPSIMD cannot touch PSUM).
                    NQ = 4
                    for quar in range(NQ):
                        bg0 = quar * (NT // (6 * NQ))
                        for bg in range(bg0, bg0 + NT // (6 * NQ)):
                            g_ps = gps.tile([128, 1024], F32, tag="g")
                            for j in range(6):
                                u = 6 * bg + j
                                col = (j // 3) * 512 + (j % 3) * CO
                                for h, XN in ((0, XN0), (1, XN1)):
                                    nc.tensor.matmul(
                                        g_ps[:, col:col + CO],
                                        XN[:, u * 128:(u + 1) * 128],
                                        vbf[:, h * CO:(h + 1) * CO],
                                        start=(h == 0), stop=(h == 1))
                            nc.scalar.copy(
                                Gb[:, bg * 6 * CO:(bg + 1) * 6 * CO]
                                .rearrange("p (s f) -> p s f", s=2),
                                g_ps[:, :].rearrange("p (s f) -> p s f",
                                                     s=2)[:, :, 0:480])
                        # P = Wg .* Gb for this u-quarter (packed bf16);
                        # first chunk of quarters 0-2 on Pool, rest on DVE
                        QW = NT // NQ * CO  # 2880 cols per quarter
                        qb = quar * QW
                        last = quar == NQ - 1
                        for hch in range(2):
                            sl = slice(qb + hch * 1440, qb + (hch + 1) * 1440)
                            eng = nc.gpsimd if (hch == 0 and not last) \
                                else nc.vector
                            eng.tensor_tensor(Wp[:, sl], Wg[:, sl],
                                              Gb[:, sl], op=ALU.mult)
                            if last:
                                warm(Wp[:, sl])
                        # o-tree for this quarter
                        o2 = QW // 2
                        Pv = Wp[:, qb:qb + QW].rearrange(
                            "p (f o) -> p f o", o=O)
                        T1v = T1[:, quar * o2:quar * o2 + o2].rearrange(
                            "p (f o) -> p f o", o=O // 2)
                        nc.vector.tensor_tensor(
                            T1v, Pv[:, :, 0:8], Pv[:, :, 8:16], op=ALU.add)
                        if last:
                            warm(T1[:, quar * o2:quar * o2 + 4])
                        o4 = QW // 4
                        T2v = T2[:, quar * o4:quar * o4 + o4].rearrange(
                            "p (f o) -> p f o", o=O // 4)
                        nc.vector.tensor_tensor(
                            T2v, T1v[:, :, 0:4], T1v[:, :, 4:8], op=ALU.add)
                        if last:
                            warm(T2[:, quar * o4:quar * o4 + 4])
                        o8 = QW // 8
                        T3v = T3[:, quar * o8:quar * o8 + o8].rearrange(
                            "p (f o) -> p f o", o=O // 8)
                        nc.vector.tensor_tensor(
                            T3v, T2v[:, :, 0:2], T2v[:, :, 2:4], op=ALU.add)
                        if last:
                            warm(T3[:, quar * o8:quar * o8 + 4])
                        o16 = QW // 16
                        nc.vector.tensor_tensor(
                            Q[:, quar * o16:quar * o16 + o16].rearrange(
                                "p (f o) -> p f o", o=1),
                            T3v[:, :, 0:1], T3v[:, :, 1:2], op=ALU.add)
                        if last:
                            warm(Q[:, quar * o16:quar * o16 + 2])
                        # this quarter covers i-planes (2q, 2q+1): fold its
                        # i-pair and accumulate into b right away so only the
                        # last quarter's chain is exposed past the G matmuls
                        qq = quar * o16
                        nc.vector.tensor_add(Q[:, qq:qq + o16 // 2],
                                             Q[:, qq:qq + o16 // 2],
                                             Q[:, qq + o16 // 2:qq + o16])
                        nc.vector.tensor_add(b_sb[:, :], b_sb[:, :],
                                             Q[:, qq:qq + o16 // 2])
                        if last:
                            warm(b_sb[:, :])

    nc.compile()
    return nc


def _host_inputs(x, W):
    # rows ordered (i, r), r-major tiles: tile u = i*RT + T holds
    # r = T*128 + p on partition p.  Free order (c, o).  Partition-major.
    wg = np.ascontiguousarray(
        W.transpose(3, 0, 1, 2).reshape(I, RT, 128, CO)
        .transpose(2, 0, 1, 3).reshape(128, NT * CO)).astype(ml_dtypes.bfloat16)
    xtf = np.ascontiguousarray(
        x.transpose(2, 1, 0).reshape(I, RT, 128, B)
        .transpose(2, 0, 1, 3).reshape(128, NT * B)).astype(ml_dtypes.bfloat16)
    xn = [np.ascontiguousarray(
        x[h * 128:(h + 1) * 128].transpose(0, 2, 1).reshape(128, NT * 128))
        .astype(ml_dtypes.bfloat16) for h in range(2)]
    in_maps = []
    for cidx in range(NCORES):
        xc = x[cidx * BL:(cidx + 1) * BL]          # (BL, R, I)
        xto = np.ascontiguousarray(
            xc.transpose(2, 1, 0).reshape(I, RT, 128, BL)
            .transpose(2, 0, 1, 3).reshape(128, NT * BL)).astype(
            ml_dtypes.bfloat16)
        in_maps.append({"wg": wg, "xtf": xtf, "xto": xto,
                        "xn0": xn[0], "xn1": xn[1]})
    return in_maps


def kernel(x, W):
    x = np.ascontiguousarray(np.asarray(x, dtype=np.float32))
    W = np.ascontiguousarray(np.asarray(W, dtype=np.float32))
    assert x.shape == (B, R, I) and W.shape == (R, C, O, I)
    if "nc" not in _CACHE:
        _CACHE["nc"] = _build()
    nc = _CACHE["nc"]
    in_maps = _host_inputs(x, W)
    res = bass_utils.run_bass_kernel_spmd(nc, in_maps,
                                          core_ids=list(range(NCORES)))
    # out [80, 2*BL]: out[p, hh*BL + b] = v[b_own, co = hh*80 + p]
    vs = []
    for r in res.results:
        o = r["out"].reshape(80, 2, BL).transpose(1, 0, 2).reshape(CO, BL)
        vs.append(o.T.reshape(BL, C, O))
    return np.concatenate(vs, axis=0)[..., None].astype(np.float32)


# revision 5
# speedup vs baseline: 1.0381x; 1.0070x over previous
"""DigitCaps (CapsNet dynamic routing) kernel for 8 Trainium2 NeuronCores.

Reference math:
  u_hat[b,r,c,o] = sum_i W[r,c,o,i] * x[b,r,i]
  b_ij = 0;  3 routing iterations:
     c = softmax_r(b);  s[b,c,o] = sum_r c[r,c] u_hat[b,r,c,o];
     v = squash(s);     b += mean_b(sum_o u_hat[b,r,c,o] v[b,c,o])
  returns v[..., None]  (256, 10, 16, 1)

Strategy: ZERO-COMMUNICATION full replication.  The routing logits b_ij are
batch-shared, so the routing trajectory is identical on every core; each
core computes it for the FULL batch (B=256) for iterations 0-1 (whose only
product is the shared b_ij update), then computes the final-iteration
capsule outputs only for its OWN 32-batch slice (per-core xto input).  No
collectives, no remote DMA, no cross-core sync of any kind.

u_hat (189 MB) is never materialized: the routing coefficients are folded
into the weights so every pass is a dense matmul over K=(i,r)=9216:
    s-matmul:  s[b,(c,o)]   = sum_K  XTF[K,b] * (c-scaled Wg)[K,(c,o)]
    G-matmul:  G[K,(c,o)]   = sum_b  XN[b,K] * (v[b,(c,o)]/B)
    agreement: abar[r,c]    = sum_{i,o} Wg .* G
Rows are ordered (i, r) with r-major tiles of 128, so partition p of tile
u=(i,T) holds r = T*128+p.  x is DMA'd in both layouts (K-major XTF for
the s-matmul, b-major XN halves for the G-matmul), chunk-interleaved in
first-use order so every pass streams behind the serial DMA device.  The
o-reduction of abar is a tree of packed bf16 adds run per u-quarter so it
pipelines into the G drains, the i-reduction paired adds per quarter, the
softmax a partition_all_reduce + free-axis reduce, and the c-fold a packed
bf16 broadcast multiply (DVE-heavy split; Pool tensor ops model ~2.5x
slower per element).  PSUM accumulation chains each own a full bank (2 KB
zero region).  Matmuls run in bf16; softmax and squash in fp32.
"""
import sys
if '/opt/trn_rl_repo' not in sys.path:
    sys.path.insert(0, '/opt/trn_rl_repo')
import numpy as np
import ml_dtypes

import concourse.bass as bass
import concourse.bacc as bacc
import concourse.mybir as mybir
import concourse.tile as tile
from concourse import bass_utils
from concourse import bass_isa

BF16 = mybir.dt.bfloat16
F32 = mybir.dt.float32
ALU = mybir.AluOpType
ACT = mybir.ActivationFunctionType

B, R, C, O, I = 256, 1152, 10, 16, 8
NCORES = 8
BL = B // NCORES          # 32 own batch (final pass only)
RT = 9                    # r tiles of 128 (per i)
NT = 72                   # (i, r) tiles of 128: u = i*RT + T
CO = C * O                # 160, free order (c,o): idx = c*O + o
NITER = 3

_CACHE = {}


def _build(n_cores=NCORES, repeat=1):
    nc = bacc.Bacc("TRN2", target_bir_lowering=False, debug=False,
                   num_devices=n_cores)
    wg_d = nc.dram_tensor("wg", [128, NT * CO], BF16, kind="ExternalInput")
    xtf_d = nc.dram_tensor("xtf", [128, NT * B], BF16, kind="ExternalInput")
    xto_d = nc.dram_tensor("xto", [128, NT * BL], BF16, kind="ExternalInput")
    xn0_d = nc.dram_tensor("xn0", [128, NT * 128], BF16, kind="ExternalInput")
    xn1_d = nc.dram_tensor("xn1", [128, NT * 128], BF16, kind="ExternalInput")
    out_d = nc.dram_tensor("out", [80, 2 * BL], F32, kind="ExternalOutput")

    with tile.TileContext(nc) as tc:
        with (
            tc.tile_pool(name="big", bufs=1) as big,
            tc.tile_pool(name="small", bufs=1) as small,
            tc.tile_pool(name="sps", bufs=1, space="PSUM") as sps,
            tc.tile_pool(name="gps", bufs=2, space="PSUM") as gps,
        ):
            Wg = big.tile([128, NT * CO], BF16, tag="Wg")
            Wp = big.tile([128, NT * CO], BF16, tag="Wp")
            XTF = big.tile([128, NT * B], BF16, tag="XTF")
            XTO = big.tile([128, NT * BL], BF16, tag="XTO")
            XN0 = big.tile([128, NT * 128], BF16, tag="XN0")
            XN1 = big.tile([128, NT * 128], BF16, tag="XN1")
            Gb = big.tile([128, NT * CO], BF16, tag="Gb")
            T1 = big.tile([128, NT * C * 8], BF16, tag="T1")
            T2 = big.tile([128, NT * C * 4], BF16, tag="T2")
            T3 = big.tile([128, NT * C * 2], BF16, tag="T3")
            Q = big.tile([128, NT * C], F32, tag="Q")

            b_sb = small.tile([128, RT * C], F32, tag="b")
            expb = small.tile([128, RT * C], F32, tag="expb")
            esum = small.tile([128, RT * C], F32, tag="esum")
            c_sb = small.tile([128, RT * C], F32, tag="c")
            c16 = small.tile([128, RT * C], BF16, tag="c16")
            crep = small.tile([128, RT * C * O], BF16, tag="crep")
            zp = small.tile([128, C], F32, tag="zp")
            zr = small.tile([128, C], F32, tag="zr")
            # squash working set, full batch: [128, (h, co)] = [128, 320]
            se = small.tile([128, 2 * CO], F32, tag="se")
            ab = small.tile([128, 2 * CO], F32, tag="ab")
            sq = small.tile([128, 2 * CO], F32, tag="sq")
            rd = small.tile([128, 2 * CO], F32, tag="rd")
            num = small.tile([128, 2 * CO], F32, tag="num")
            vv = small.tile([128, 2 * CO], F32, tag="vv")
            vbf = small.tile([128, 2 * CO], BF16, tag="vbf")

            for _rep in range(repeat):
                # --- input DMAs, interleaved by first use (DMA dev serializes)
                NWC = NT // 4          # Wg chunk: 18 tiles
                NNC = NT // 4          # XN chunk: 18 tiles
                for ch in range(4):
                    nc.sync.dma_start(
                        out=Wg[:, ch * NWC * CO:(ch + 1) * NWC * CO],
                        in_=wg_d[:, ch * NWC * CO:(ch + 1) * NWC * CO])
                    nc.sync.dma_start(
                        out=XTF[:, ch * NWC * B:(ch + 1) * NWC * B],
                        in_=xtf_d[:, ch * NWC * B:(ch + 1) * NWC * B])
                for ch in range(4):
                    sl = slice(ch * NNC * 128, (ch + 1) * NNC * 128)
                    nc.sync.dma_start(out=XN0[:, sl], in_=xn0_d[:, sl])
                    nc.sync.dma_start(out=XN1[:, sl], in_=xn1_d[:, sl])
                nc.sync.dma_start(out=XTO[:, :], in_=xto_d[:, :])
                nc.vector.memset(b_sb[:, :], 0.0)

                # --- persistent PSUM tiles (one bank per accumulation chain)
                sh0 = sps.tile([128, CO], F32, tag="sh0")
                sh1 = sps.tile([128, CO], F32, tag="sh1")
                s_h = [sh0, sh1]
                warm_ps = sps.tile([128, 4], F32, tag="warm")

                def warm(src, p=128):
                    """Keep the PE p-state ramped through engine-idle windows:
                    a 4-row dummy matmul whose moving operand is the output of
                    the op that gates the next real PE work."""
                    if src.dtype != BF16:
                        src = src.bitcast(BF16)
                    nc.tensor.matmul(warm_ps[0:1, 0:4], Wg[0:p, 0:1],
                                     src[0:p, 0:4], start=True, stop=True)

                for k in range(NITER):
                    if k > 0:
                        # c = softmax over r (partitions x RT tiles)
                        nc.scalar.activation(expb[:, :], b_sb[:, :], ACT.Exp)
                        warm(expb[:, :])
                        nc.gpsimd.partition_all_reduce(
                            esum[:, :], expb[:, :], channels=128,
                            reduce_op=bass_isa.ReduceOp.add)
                        warm(esum[:, :])
                        nc.vector.tensor_reduce(
                            zp[:, :],
                            esum[:, :].rearrange("p (T c) -> p c T", c=C),
                            axis=mybir.AxisListType.X, op=ALU.add)
                        warm(zp[:, :])
                        nc.vector.reciprocal(zr[:, :], zp[:, :])
                        nc.vector.tensor_tensor(
                            c_sb[:, :].rearrange("p (T c) -> p T c", c=C),
                            expb[:, :].rearrange("p (T c) -> p T c", c=C),
                            zr[:, :].unsqueeze(1).broadcast_to((128, RT, C)),
                            op=ALU.mult)
                        warm(c_sb[:, :])
                        nc.vector.tensor_copy(c16[:, :], c_sb[:, :])
                        # crep[p,(T,c,o)] = c16[p,(T,c)] replicated over o
                        nc.vector.tensor_copy(
                            crep[:, :].rearrange("p (T c o) -> p T c o",
                                                 c=C, o=O),
                            c16[:, :].rearrange("p (T c) -> p T c", c=C)
                            .unsqueeze(3).broadcast_to((128, RT, C, O)))
                        warm(crep[:, :])
                        # W' = Wg * crep, chunked per i so the s-matmul can
                        # stream behind it; i 0-6 on DVE, 7 on Pool (Pool
                        # tensor ops model ~2.5x slower per element).
                        FI = RT * C * O  # 1440, one i-plane
                        for ii in range(I):
                            eng = nc.gpsimd if ii == 5 else nc.vector
                            eng.tensor_tensor(
                                Wp[:, ii * FI:(ii + 1) * FI],
                                Wg[:, ii * FI:(ii + 1) * FI],
                                crep[:, :], op=ALU.mult)
                            if ii == 0:
                                warm(Wp[:, 0:4])

                    mov = Wg if k == 0 else Wp

                    if k < NITER - 1:
                        # s matmul, full batch: out [b-half, co], 1 bank each
                        for u in range(NT):
                            for h in range(2):
                                nc.tensor.matmul(
                                    s_h[h][:, :],
                                    XTF[:, u * B + h * 128:u * B + h * 128 + 128],
                                    mov[:, u * CO:(u + 1) * CO],
                                    start=(u == 0), stop=(u == NT - 1))
                        P, width = 128, 2 * CO
                        sq_src = [(s_h[0][:, :], 0, CO), (s_h[1][:, :], CO, CO)]
                    else:
                        # final pass: own 32 batches only, swapped orientation
                        for u in range(NT):
                            for hh in range(2):
                                nc.tensor.matmul(
                                    s_h[hh][0:80, 0:BL],
                                    mov[:, u * CO + hh * 80:u * CO + hh * 80 + 80],
                                    XTO[:, u * BL:(u + 1) * BL],
                                    start=(u == 0), stop=(u == NT - 1))
                        P, width = 80, 2 * BL
                        sq_src = [(s_h[0][0:80, 0:BL], 0, BL),
                                  (s_h[1][0:80, 0:BL], BL, BL)]

                    # squash: v = s*|s| / (1+s^2); the two PSUM reads go
                    # to different engines so they run concurrently
                    scl = 1.0 / R if k == 0 else 1.0
                    (src0, off0, w0), (src1, off1, w1) = sq_src
                    nc.scalar.activation(se[0:P, off0:off0 + w0], src0,
                                         ACT.Copy, scale=scl)
                    if scl == 1.0:
                        nc.vector.tensor_copy(se[0:P, off1:off1 + w1], src1)
                    else:
                        nc.vector.tensor_scalar_mul(se[0:P, off1:off1 + w1],
                                                    src1, scl)
                    warm(se[:, :], p=P)
                    nc.scalar.activation(ab[0:P, 0:width], se[0:P, 0:width],
                                         ACT.Abs)
                    warm(ab[:, :], p=P)
                    nc.vector.tensor_mul(sq[0:P, 0:width], se[0:P, 0:width],
                                         se[0:P, 0:width])
                    warm(sq[:, :], p=P)
                    nc.vector.tensor_scalar_add(sq[0:P, 0:width],
                                                sq[0:P, 0:width], 1.0)
                    nc.vector.reciprocal(rd[0:P, 0:width], sq[0:P, 0:width])
                    warm(rd[:, :], p=P)
                    nc.vector.tensor_mul(num[0:P, 0:width], se[0:P, 0:width],
                                         ab[0:P, 0:width])
                    warm(num[:, :], p=P)
                    nc.vector.tensor_mul(vv[0:P, 0:width], num[0:P, 0:width],
                                         rd[0:P, 0:width])
                    warm(vv[:, :], p=P)

                    if k == NITER - 1:
                        nc.sync.dma_start(out=out_d[:, :], in_=vv[0:80, 0:width])
                        continue

                    nc.scalar.activation(vbf[:, :], vv[:, :], ACT.Copy,
                                         scale=1.0 / B)
                    warm(vbf[:, :])

                    # G matmul over b (2 chained halves per region); PSUM
                    # drains to bf16 round-robin ACT/Pool; the agreement
                    # P-mult + o-tree runs per u-quarter so only the last
                    # quarter's tree is exposed past the final G matmul.
                    # 12 big-groups of 6 K-tiles; each lands in a 2-bank PSUM
                    # tile (3 regions per bank, 480+pad layout) and drains in
                    # one strided ACT copy (GPSIMD cannot touch PSUM).
                    # Uneven chunks (in big-groups of 6 K-tiles): the early
                    # ones hide under the G matmul stream; the last is a
                    # single big-group so the exposed tree tail is short.
                    BOUNDS = [(0, 3), (3, 6), (6, 9), (9, 12)]
                    for ci, (b0, b1) in enumerate(BOUNDS):
                        for bg in range(b0, b1):
                            g_ps = gps.tile([128, 1024], F32, tag="g")
                            for j in range(6):
                                u = 6 * bg + j
                                col = (j // 3) * 512 + (j % 3) * CO
                                for h, XN in ((0, XN0), (1, XN1)):
                                    nc.tensor.matmul(
                                        g_ps[:, col:col + CO],
                                        XN[:, u * 128:(u + 1) * 128],
                                        vbf[:, h * CO:(h + 1) * CO],
                                        start=(h == 0), stop=(h == 1))
                            nc.scalar.copy(
                                Gb[:, bg * 6 * CO:(bg + 1) * 6 * CO]
                                .rearrange("p (s f) -> p s f", s=2),
                                g_ps[:, :].rearrange("p (s f) -> p s f",
                                                     s=2)[:, :, 0:480])
                        # P = Wg .* Gb for this chunk (packed bf16); one
                        # half of the two big chunks goes to Pool
                        qb = b0 * 6 * CO
                        QW = (b1 - b0) * 6 * CO
                        last = ci == len(BOUNDS) - 1
                        nch = 2 if QW > 1920 else 1
                        for hch in range(nch):
                            w = QW // nch
                            sl = slice(qb + hch * w, qb + (hch + 1) * w)
                            eng = nc.gpsimd if (hch == 0 and ci < 3) \
                                else nc.vector
                            eng.tensor_tensor(Wp[:, sl], Wg[:, sl],
                                              Gb[:, sl], op=ALU.mult)
                            if last:
                                warm(Wp[:, sl])
                        # o-tree for this chunk
                        o2 = QW // 2
                        Pv = Wp[:, qb:qb + QW].rearrange(
                            "p (f o) -> p f o", o=O)
                        T1v = T1[:, qb // 2:qb // 2 + o2].rearrange(
                            "p (f o) -> p f o", o=O // 2)
                        nc.vector.tensor_tensor(
                            T1v, Pv[:, :, 0:8], Pv[:, :, 8:16], op=ALU.add)
                        if last:
                            warm(T1[:, qb // 2:qb // 2 + 4])
                        o4 = QW // 4
                        T2v = T2[:, qb // 4:qb // 4 + o4].rearrange(
                            "p (f o) -> p f o", o=O // 4)
                        nc.vector.tensor_tensor(
                            T2v, T1v[:, :, 0:4], T1v[:, :, 4:8], op=ALU.add)
                        if last:
                            warm(T2[:, qb // 4:qb // 4 + 4])
                        o8 = QW // 8
                        T3v = T3[:, qb // 8:qb // 8 + o8].rearrange(
                            "p (f o) -> p f o", o=O // 8)
                        nc.vector.tensor_tensor(
                            T3v, T2v[:, :, 0:2], T2v[:, :, 2:4], op=ALU.add)
                        if last:
                            warm(T3[:, qb // 8:qb // 8 + 4])
                        o16 = QW // 16
                        nc.vector.tensor_tensor(
                            Q[:, qb // 16:qb // 16 + o16].rearrange(
                                "p (f o) -> p f o", o=1),
                            T3v[:, :, 0:1], T3v[:, :, 1:2], op=ALU.add)
                        if last:
                            warm(Q[:, qb // 16:qb // 16 + 2])
                        # each chunk covers exactly 2 i-planes: fold the
                        # pair and accumulate into b right away so only the
                        # last chunk's chain is exposed past the G matmuls
                        qq, w2 = qb // 16, o16 // 2
                        nc.vector.tensor_add(Q[:, qq:qq + w2],
                                             Q[:, qq:qq + w2],
                                             Q[:, qq + w2:qq + 2 * w2])
                        nc.vector.tensor_add(b_sb[:, :], b_sb[:, :],
                                             Q[:, qq:qq + w2])
                        if last:
                            warm(b_sb[:, :])



    nc.compile()
    return nc


def _host_inputs(x, W):
    # rows ordered (i, r), r-major tiles: tile u = i*RT + T holds
    # r = T*128 + p on partition p.  Free order (c, o).  Partition-major.
    wg = np.ascontiguousarray(
        W.transpose(3, 0, 1, 2).reshape(I, RT, 128, CO)
        .transpose(2, 0, 1, 3).reshape(128, NT * CO)).astype(ml_dtypes.bfloat16)
    xtf = np.ascontiguousarray(
        x.transpose(2, 1, 0).reshape(I, RT, 128, B)
        .transpose(2, 0, 1, 3).reshape(128, NT * B)).astype(ml_dtypes.bfloat16)
    xn = [np.ascontiguousarray(
        x[h * 128:(h + 1) * 128].transpose(0, 2, 1).reshape(128, NT * 128))
        .astype(ml_dtypes.bfloat16) for h in range(2)]
    in_maps = []
    for cidx in range(NCORES):
        xc = x[cidx * BL:(cidx + 1) * BL]          # (BL, R, I)
        xto = np.ascontiguousarray(
            xc.transpose(2, 1, 0).reshape(I, RT, 128, BL)
            .transpose(2, 0, 1, 3).reshape(128, NT * BL)).astype(
            ml_dtypes.bfloat16)
        in_maps.append({"wg": wg, "xtf": xtf, "xto": xto,
                        "xn0": xn[0], "xn1": xn[1]})
    return in_maps


def kernel(x, W):
    x = np.ascontiguousarray(np.asarray(x, dtype=np.float32))
    W = np.ascontiguousarray(np.asarray(W, dtype=np.float32))
    assert x.shape == (B, R, I) and W.shape == (R, C, O, I)
    if "nc" not in _CACHE:
        _CACHE["nc"] = _build()
    nc = _CACHE["nc"]
    in_maps = _host_inputs(x, W)
    res = bass_utils.run_bass_kernel_spmd(nc, in_maps,
                                          core_ids=list(range(NCORES)))
    # out [80, 2*BL]: out[p, hh*BL + b] = v[b_own, co = hh*80 + p]
    vs = []
    for r in res.results:
        o = r["out"].reshape(80, 2, BL).transpose(1, 0, 2).reshape(CO, BL)
        vs.append(o.T.reshape(BL, C, O))
    return np.concatenate(vs, axis=0)[..., None].astype(np.float32)


# revision 6
# speedup vs baseline: 1.0459x; 1.0076x over previous
"""DigitCaps (CapsNet dynamic routing) kernel for 8 Trainium2 NeuronCores.

Reference math:
  u_hat[b,r,c,o] = sum_i W[r,c,o,i] * x[b,r,i]
  b_ij = 0;  3 routing iterations:
     c = softmax_r(b);  s[b,c,o] = sum_r c[r,c] u_hat[b,r,c,o];
     v = squash(s);     b += mean_b(sum_o u_hat[b,r,c,o] v[b,c,o])
  returns v[..., None]  (256, 10, 16, 1)

Strategy: ZERO-COMMUNICATION full replication.  The routing logits b_ij are
batch-shared, so the routing trajectory is identical on every core; each
core computes it for the FULL batch (B=256) for iterations 0-1 (whose only
product is the shared b_ij update), then computes the final-iteration
capsule outputs only for its OWN 32-batch slice (per-core xto input).  No
collectives, no remote DMA, no cross-core sync of any kind.

u_hat (189 MB) is never materialized: the routing coefficients are folded
into the weights so every pass is a dense matmul over K=(i,r)=9216:
    s-matmul:  s[b,(c,o)]   = sum_K  XTF[K,b] * (c-scaled Wg)[K,(c,o)]
    G-matmul:  G[K,(c,o)]   = sum_b  XN[b,K] * (v[b,(c,o)]/B)
    agreement: abar[r,c]    = sum_{i,o} Wg .* G
Rows are ordered (i, r) with r-major tiles of 128, so partition p of tile
u=(i,T) holds r = T*128+p.  x is DMA'd in both layouts (K-major XTF for
the s-matmul, b-major XN halves for the G-matmul), chunk-interleaved in
first-use order so every pass streams behind the serial DMA device.  The
o-reduction of abar is a tree of packed bf16 adds run per u-quarter so it
pipelines into the G drains, the i-reduction paired adds per quarter, the
softmax a partition_all_reduce + free-axis reduce, and the c-fold a packed
bf16 broadcast multiply (DVE-heavy split; Pool tensor ops model ~2.5x
slower per element).  PSUM accumulation chains each own a full bank (2 KB
zero region).  Matmuls run in bf16; softmax and squash in fp32.
"""
import sys
if '/opt/trn_rl_repo' not in sys.path:
    sys.path.insert(0, '/opt/trn_rl_repo')
import numpy as np
import ml_dtypes

import concourse.bass as bass
import concourse.bacc as bacc
import concourse.mybir as mybir
import concourse.tile as tile
from concourse import bass_utils
from concourse import bass_isa

BF16 = mybir.dt.bfloat16
F32 = mybir.dt.float32
ALU = mybir.AluOpType
ACT = mybir.ActivationFunctionType

B, R, C, O, I = 256, 1152, 10, 16, 8
NCORES = 8
BL = B // NCORES          # 32 own batch (final pass only)
RT = 9                    # r tiles of 128 (per i)
NT = 72                   # (i, r) tiles of 128: u = i*RT + T
CO = C * O                # 160, free order (c,o): idx = c*O + o
NITER = 3

_CACHE = {}


def _build(n_cores=NCORES, repeat=1):
    nc = bacc.Bacc("TRN2", target_bir_lowering=False, debug=False,
                   num_devices=n_cores)
    wg_d = nc.dram_tensor("wg", [128, NT * CO], BF16, kind="ExternalInput")
    xtf_d = nc.dram_tensor("xtf", [128, NT * B], BF16, kind="ExternalInput")
    xto_d = nc.dram_tensor("xto", [128, NT * BL], BF16, kind="ExternalInput")
    xn0_d = nc.dram_tensor("xn0", [128, NT * 128], BF16, kind="ExternalInput")
    xn1_d = nc.dram_tensor("xn1", [128, NT * 128], BF16, kind="ExternalInput")
    out_d = nc.dram_tensor("out", [80, 2 * BL], F32, kind="ExternalOutput")

    with tile.TileContext(nc) as tc:
        with (
            tc.tile_pool(name="big", bufs=1) as big,
            tc.tile_pool(name="small", bufs=1) as small,
            tc.tile_pool(name="sps", bufs=1, space="PSUM") as sps,
            tc.tile_pool(name="gps", bufs=2, space="PSUM") as gps,
        ):
            Wg = big.tile([128, NT * CO], BF16, tag="Wg")
            Wp = big.tile([128, NT * CO], BF16, tag="Wp")
            XTF = big.tile([128, NT * B], BF16, tag="XTF")
            XTO = big.tile([128, NT * BL], BF16, tag="XTO")
            XN0 = big.tile([128, NT * 128], BF16, tag="XN0")
            XN1 = big.tile([128, NT * 128], BF16, tag="XN1")
            Gb = big.tile([128, NT * CO], BF16, tag="Gb")
            T1 = big.tile([128, NT * C * 8], BF16, tag="T1")
            T2 = big.tile([128, NT * C * 4], BF16, tag="T2")
            T3 = big.tile([128, NT * C * 2], BF16, tag="T3")
            Q = big.tile([128, NT * C], F32, tag="Q")

            b_sb = small.tile([128, RT * C], F32, tag="b")
            expb = small.tile([128, RT * C], F32, tag="expb")
            esum = small.tile([128, RT * C], F32, tag="esum")
            c_sb = small.tile([128, RT * C], F32, tag="c")
            c16 = small.tile([128, RT * C], BF16, tag="c16")
            crep = small.tile([128, RT * C * O], BF16, tag="crep")
            zp = small.tile([128, C], F32, tag="zp")
            zr = small.tile([128, C], F32, tag="zr")
            # squash working set, full batch: [128, (h, co)] = [128, 320]
            se = small.tile([128, 2 * CO], F32, tag="se")
            ab = small.tile([128, 2 * CO], F32, tag="ab")
            sq = small.tile([128, 2 * CO], F32, tag="sq")
            rd = small.tile([128, 2 * CO], F32, tag="rd")
            num = small.tile([128, 2 * CO], F32, tag="num")
            vv = small.tile([128, 2 * CO], F32, tag="vv")
            vbf = small.tile([128, 2 * CO], BF16, tag="vbf")

            for _rep in range(repeat):
                # --- input DMAs, interleaved by first use (DMA dev serializes)
                NWC = NT // 8          # Wg chunk: 9 tiles
                NNC = NT // 4          # XN chunk: 18 tiles
                for ch in range(8):
                    nc.sync.dma_start(
                        out=Wg[:, ch * NWC * CO:(ch + 1) * NWC * CO],
                        in_=wg_d[:, ch * NWC * CO:(ch + 1) * NWC * CO])
                    nc.sync.dma_start(
                        out=XTF[:, ch * NWC * B:(ch + 1) * NWC * B],
                        in_=xtf_d[:, ch * NWC * B:(ch + 1) * NWC * B])
                for ch in range(4):
                    sl = slice(ch * NNC * 128, (ch + 1) * NNC * 128)
                    nc.sync.dma_start(out=XN0[:, sl], in_=xn0_d[:, sl])
                    nc.sync.dma_start(out=XN1[:, sl], in_=xn1_d[:, sl])
                nc.sync.dma_start(out=XTO[:, :], in_=xto_d[:, :])
                nc.vector.memset(b_sb[:, :], 0.0)

                # --- persistent PSUM tiles (one bank per accumulation chain)
                sh0 = sps.tile([128, CO], F32, tag="sh0")
                sh1 = sps.tile([128, CO], F32, tag="sh1")
                s_h = [sh0, sh1]
                warm_ps = sps.tile([128, 4], F32, tag="warm")

                def warm(src, p=128):
                    """Keep the PE p-state ramped through engine-idle windows:
                    a 4-row dummy matmul whose moving operand is the output of
                    the op that gates the next real PE work."""
                    if src.dtype != BF16:
                        src = src.bitcast(BF16)
                    nc.tensor.matmul(warm_ps[0:1, 0:4], Wg[0:p, 0:1],
                                     src[0:p, 0:4], start=True, stop=True)

                for k in range(NITER):
                    if k > 0:
                        # c = softmax over r (partitions x RT tiles)
                        nc.scalar.activation(expb[:, :], b_sb[:, :], ACT.Exp)
                        warm(expb[:, :])
                        nc.gpsimd.partition_all_reduce(
                            esum[:, :], expb[:, :], channels=128,
                            reduce_op=bass_isa.ReduceOp.add)
                        warm(esum[:, :])
                        nc.vector.tensor_reduce(
                            zp[:, :],
                            esum[:, :].rearrange("p (T c) -> p c T", c=C),
                            axis=mybir.AxisListType.X, op=ALU.add)
                        warm(zp[:, :])
                        nc.vector.reciprocal(zr[:, :], zp[:, :])
                        nc.vector.tensor_tensor(
                            c_sb[:, :].rearrange("p (T c) -> p T c", c=C),
                            expb[:, :].rearrange("p (T c) -> p T c", c=C),
                            zr[:, :].unsqueeze(1).broadcast_to((128, RT, C)),
                            op=ALU.mult)
                        warm(c_sb[:, :])
                        nc.vector.tensor_copy(c16[:, :], c_sb[:, :])
                        # crep[p,(T,c,o)] = c16[p,(T,c)] replicated over o
                        nc.vector.tensor_copy(
                            crep[:, :].rearrange("p (T c o) -> p T c o",
                                                 c=C, o=O),
                            c16[:, :].rearrange("p (T c) -> p T c", c=C)
                            .unsqueeze(3).broadcast_to((128, RT, C, O)))
                        warm(crep[:, :])
                        # W' = Wg * crep, chunked per i so the s-matmul can
                        # stream behind it; i 0-6 on DVE, 7 on Pool (Pool
                        # tensor ops model ~2.5x slower per element).
                        FI = RT * C * O  # 1440, one i-plane
                        for ii in range(I):
                            eng = nc.gpsimd if ii == 5 else nc.vector
                            eng.tensor_tensor(
                                Wp[:, ii * FI:(ii + 1) * FI],
                                Wg[:, ii * FI:(ii + 1) * FI],
                                crep[:, :], op=ALU.mult)
                            if ii == 0:
                                warm(Wp[:, 0:4])

                    mov = Wg if k == 0 else Wp

                    if k < NITER - 1:
                        # s matmul, full batch: out [b-half, co], 1 bank each
                        for u in range(NT):
                            for h in range(2):
                                nc.tensor.matmul(
                                    s_h[h][:, :],
                                    XTF[:, u * B + h * 128:u * B + h * 128 + 128],
                                    mov[:, u * CO:(u + 1) * CO],
                                    start=(u == 0), stop=(u == NT - 1))
                        P, width = 128, 2 * CO
                        sq_src = [(s_h[0][:, :], 0, CO), (s_h[1][:, :], CO, CO)]
                    else:
                        # final pass: own 32 batches only, swapped orientation
                        for u in range(NT):
                            for hh in range(2):
                                nc.tensor.matmul(
                                    s_h[hh][0:80, 0:BL],
                                    mov[:, u * CO + hh * 80:u * CO + hh * 80 + 80],
                                    XTO[:, u * BL:(u + 1) * BL],
                                    start=(u == 0), stop=(u == NT - 1))
                        P, width = 80, 2 * BL
                        sq_src = [(s_h[0][0:80, 0:BL], 0, BL),
                                  (s_h[1][0:80, 0:BL], BL, BL)]

                    # squash: v = s*|s| / (1+s^2); the two PSUM reads go
                    # to different engines so they run concurrently
                    scl = 1.0 / R if k == 0 else 1.0
                    (src0, off0, w0), (src1, off1, w1) = sq_src
                    nc.scalar.activation(se[0:P, off0:off0 + w0], src0,
                                         ACT.Copy, scale=scl)
                    if scl == 1.0:
                        nc.vector.tensor_copy(se[0:P, off1:off1 + w1], src1)
                    else:
                        nc.vector.tensor_scalar_mul(se[0:P, off1:off1 + w1],
                                                    src1, scl)
                    warm(se[:, :], p=P)
                    nc.scalar.activation(ab[0:P, 0:width], se[0:P, 0:width],
                                         ACT.Abs)
                    warm(ab[:, :], p=P)
                    nc.vector.tensor_mul(sq[0:P, 0:width], se[0:P, 0:width],
                                         se[0:P, 0:width])
                    warm(sq[:, :], p=P)
                    nc.vector.tensor_scalar_add(sq[0:P, 0:width],
                                                sq[0:P, 0:width], 1.0)
                    nc.vector.reciprocal(rd[0:P, 0:width], sq[0:P, 0:width])
                    warm(rd[:, :], p=P)
                    nc.gpsimd.tensor_tensor(num[0:P, 0:width],
                                            se[0:P, 0:width],
                                            ab[0:P, 0:width], op=ALU.mult)
                    warm(num[:, :], p=P)
                    nc.vector.tensor_mul(vv[0:P, 0:width], num[0:P, 0:width],
                                         rd[0:P, 0:width])
                    warm(vv[:, :], p=P)

                    if k == NITER - 1:
                        nc.sync.dma_start(out=out_d[:, :], in_=vv[0:80, 0:width])
                        continue

                    for hh2 in range(2):
                        nc.scalar.activation(vbf[:, hh2 * CO:(hh2 + 1) * CO],
                                             vv[:, hh2 * CO:(hh2 + 1) * CO],
                                             ACT.Copy, scale=1.0 / B)
                    warm(vbf[:, :])

                    # G matmul over b (2 chained halves per region); PSUM
                    # drains to bf16 round-robin ACT/Pool; the agreement
                    # P-mult + o-tree runs per u-quarter so only the last
                    # quarter's tree is exposed past the final G matmul.
                    # 12 big-groups of 6 K-tiles; each lands in a 2-bank PSUM
                    # tile (3 regions per bank, 480+pad layout) and drains in
                    # one strided ACT copy (GPSIMD cannot touch PSUM).
                    # Uneven chunks (in big-groups of 6 K-tiles): the early
                    # ones hide under the G matmul stream; the last is a
                    # single big-group so the exposed tree tail is short.
                    BOUNDS = [(0, 3), (3, 6), (6, 9), (9, 12)]
                    for ci, (b0, b1) in enumerate(BOUNDS):
                        for bg in range(b0, b1):
                            g_ps = gps.tile([128, 1024], F32, tag="g")
                            for j in range(6):
                                u = 6 * bg + j
                                col = (j // 3) * 512 + (j % 3) * CO
                                for h, XN in ((0, XN0), (1, XN1)):
                                    nc.tensor.matmul(
                                        g_ps[:, col:col + CO],
                                        XN[:, u * 128:(u + 1) * 128],
                                        vbf[:, h * CO:(h + 1) * CO],
                                        start=(h == 0), stop=(h == 1))
                            nc.scalar.copy(
                                Gb[:, bg * 6 * CO:(bg + 1) * 6 * CO]
                                .rearrange("p (s f) -> p s f", s=2),
                                g_ps[:, :].rearrange("p (s f) -> p s f",
                                                     s=2)[:, :, 0:480])
                        # P = Wg .* Gb for this chunk (packed bf16); one
                        # half of the two big chunks goes to Pool
                        qb = b0 * 6 * CO
                        QW = (b1 - b0) * 6 * CO
                        last = ci == len(BOUNDS) - 1
                        nch = 3 if last else 2
                        for hch in range(nch):
                            w = QW // nch
                            sl = slice(qb + hch * w, qb + (hch + 1) * w)
                            eng = nc.gpsimd if (hch == 0 and not last) \
                                else nc.vector
                            eng.tensor_tensor(Wp[:, sl], Wg[:, sl],
                                              Gb[:, sl], op=ALU.mult)
                            if last:
                                warm(Wp[:, sl])
                        # o-tree for this chunk
                        o2 = QW // 2
                        Pv = Wp[:, qb:qb + QW].rearrange(
                            "p (f o) -> p f o", o=O)
                        T1v = T1[:, qb // 2:qb // 2 + o2].rearrange(
                            "p (f o) -> p f o", o=O // 2)
                        nc.vector.tensor_tensor(
                            T1v, Pv[:, :, 0:8], Pv[:, :, 8:16], op=ALU.add)
                        if last:
                            warm(T1[:, qb // 2:qb // 2 + 4])
                        o4 = QW // 4
                        T2v = T2[:, qb // 4:qb // 4 + o4].rearrange(
                            "p (f o) -> p f o", o=O // 4)
                        nc.vector.tensor_tensor(
                            T2v, T1v[:, :, 0:4], T1v[:, :, 4:8], op=ALU.add)
                        if last:
                            warm(T2[:, qb // 4:qb // 4 + 4])
                        o8 = QW // 8
                        T3v = T3[:, qb // 8:qb // 8 + o8].rearrange(
                            "p (f o) -> p f o", o=O // 8)
                        nc.vector.tensor_tensor(
                            T3v, T2v[:, :, 0:2], T2v[:, :, 2:4], op=ALU.add)
                        if last:
                            warm(T3[:, qb // 8:qb // 8 + 4])
                        o16 = QW // 16
                        nc.vector.tensor_tensor(
                            Q[:, qb // 16:qb // 16 + o16].rearrange(
                                "p (f o) -> p f o", o=1),
                            T3v[:, :, 0:1], T3v[:, :, 1:2], op=ALU.add)
                        if last:
                            warm(Q[:, qb // 16:qb // 16 + 2])
                        # each chunk covers exactly 2 i-planes: fold the
                        # pair and accumulate into b right away so only the
                        # last chunk's chain is exposed past the G matmuls
                        qq, w2 = qb // 16, o16 // 2
                        nc.vector.tensor_add(Q[:, qq:qq + w2],
                                             Q[:, qq:qq + w2],
                                             Q[:, qq + w2:qq + 2 * w2])
                        nc.vector.tensor_add(b_sb[:, :], b_sb[:, :],
                                             Q[:, qq:qq + w2])
                        if last:
                            warm(b_sb[:, :])



    nc.compile()
    return nc


def _host_inputs(x, W):
    # rows ordered (i, r), r-major tiles: tile u = i*RT + T holds
    # r = T*128 + p on partition p.  Free order (c, o).  Partition-major.
    wg = np.ascontiguousarray(
        W.transpose(3, 0, 1, 2).reshape(I, RT, 128, CO)
        .transpose(2, 0, 1, 3).reshape(128, NT * CO)).astype(ml_dtypes.bfloat16)
    xtf = np.ascontiguousarray(
        x.transpose(2, 1, 0).reshape(I, RT, 128, B)
        .transpose(2, 0, 1, 3).reshape(128, NT * B)).astype(ml_dtypes.bfloat16)
    xn = [np.ascontiguousarray(
        x[h * 128:(h + 1) * 128].transpose(0, 2, 1).reshape(128, NT * 128))
        .astype(ml_dtypes.bfloat16) for h in range(2)]
    in_maps = []
    for cidx in range(NCORES):
        xc = x[cidx * BL:(cidx + 1) * BL]          # (BL, R, I)
        xto = np.ascontiguousarray(
            xc.transpose(2, 1, 0).reshape(I, RT, 128, BL)
            .transpose(2, 0, 1, 3).reshape(128, NT * BL)).astype(
            ml_dtypes.bfloat16)
        in_maps.append({"wg": wg, "xtf": xtf, "xto": xto,
                        "xn0": xn[0], "xn1": xn[1]})
    return in_maps


def kernel(x, W):
    x = np.ascontiguousarray(np.asarray(x, dtype=np.float32))
    W = np.ascontiguousarray(np.asarray(W, dtype=np.float32))
    assert x.shape == (B, R, I) and W.shape == (R, C, O, I)
    if "nc" not in _CACHE:
        _CACHE["nc"] = _build()
    nc = _CACHE["nc"]
    in_maps = _host_inputs(x, W)
    res = bass_utils.run_bass_kernel_spmd(nc, in_maps,
                                          core_ids=list(range(NCORES)))
    # out [80, 2*BL]: out[p, hh*BL + b] = v[b_own, co = hh*80 + p]
    vs = []
    for r in res.results:
        o = r["out"].reshape(80, 2, BL).transpose(1, 0, 2).reshape(CO, BL)
        vs.append(o.T.reshape(BL, C, O))
    return np.concatenate(vs, axis=0)[..., None].astype(np.float32)
